# revision 8
# baseline (speedup 1.0000x reference)
"""Trainium2 Bass kernel for DescriptorNetwork (Roost-style GNN message passing).

Structure exploited (verified at runtime in kernel()):
  - N = C*K nodes, K=5 elements per crystal, edges = all-pairs within crystal
  - self_fea_idx = repeat(arange(N), 5), nbr_fea_idx = per-crystal tile,
    cry_elem_idx = repeat(arange(C), 5)
  => every gather is a strided/broadcast access pattern; every segment
     reduction is over 5 contiguous elements.

Sharding: 1250 crystals per core x 8 cores, fully data parallel, no
collectives.  Everything on-chip is feature-major (features on SBUF
partitions, nodes/edges along the free dimension).

v2 layout (vs the first working version):
  - elem_fea is transposed host-side -> no on-chip transposes for embedding
  - x keeps an fp32 master (xT) plus a bf16 mirror (xb); catT is built once
    per layer into a persistent bf16 cache and reused by both passes
  - Lrelu activations are split between the scalar engine (ACT) and the
    vector engine (mul+max 2-op form; biases are all zero per the spec)
  - gate broadcast to 64 partitions via a K=1 ones-matmul on the PE from a
    bf16 copy of the normalized gate (no DRAM bounce)
  - pooling matmuls run in bf16 from xb
  - output stays feature-major [64, c_s]; the host transposes
  - residual updates are chunked so the next layer starts while the tail of
    the current layer finishes (keeps the PE HAM clock warm)
"""

import numpy as np
import ml_dtypes
from contextlib import ExitStack

import concourse.bass as bass
import concourse.tile as tile
from concourse import mybir
from concourse.alu_op_type import AluOpType
from concourse.bass_utils import run_bass_kernel_spmd

FP32 = mybir.dt.float32
BF16 = mybir.dt.bfloat16
AF = mybir.ActivationFunctionType

# Model constants (hardcoded per problem spec)
C_TOT = 10000
K = 5
N_TOT = C_TOT * K
EMB = 200
F = 64
L = 3
H = 3
HID = 256
NCORES = 8

C_S = C_TOT // NCORES          # 1250 crystals per core
GCOLS = 250                    # gate buffer: 250 edges (10 crystals) per row
WCOLS = 50                     # node buffer: 50 nodes (10 crystals) per row
TE = 500                       # edge tile (100 nodes, 20 crystals)
TN = 500                       # node tile for pooling
TNE = 512                      # node tile for embedding
RES_GROUP = 16                 # pass-2 tiles per residual-update chunk

# Lrelu engine per (head, chunk) slot: 's'=scalar ACT, 'v'=vector 2-op
ENG_P1 = ["s", "v", "s", "s", "v", "s"]
ENG_P2 = ["s", "v", "s", "s", "v", "s"]


def _tiles(total, size):
    out, o = [], 0
    while o < total:
        out.append((o, min(size, total - o)))
        o += size
    return out


def _split_multiwaits(nc):
    """Walrus in this container encodes at most one on_wait per instruction;
    Tile emits several.  Split extras into preceding wait-only instructions."""
    n_split = 0
    for bb in nc.main_func.blocks:
        new = []
        for inst in bb.instructions:
            si = getattr(inst, "sync_info", None)
            waits = list(si.on_wait) if (si is not None and si.on_wait) else []
            if len(waits) > 1:
                for w in waits[:-1]:
                    ev = mybir.InstEventSemaphore(
                        name=f"{inst.name}-w{n_split}",
                        ins=[], outs=[],
                        sync_info=mybir.SyncInfo(on_wait=[w], on_update=[]),
                    )
                    ev.engine = inst.engine
                    new.append(ev)
                    n_split += 1
                si.on_wait = [waits[-1]]
            new.append(inst)
        bb.instructions[:] = new
    return n_split


def build_bass(c_s=C_S, split_waits=True):
    """Build the per-core Bass program (same program on all 8 cores).
    Assumes all hidden-layer biases (b1) are zero (guaranteed by the spec
    fills; checked host-side with fallback)."""
    n_s, e_s = c_s * K, c_s * K * K
    assert e_s % GCOLS == 0 and n_s % WCOLS == 0
    grows, wrows = e_s // GCOLS, n_s // WCOLS

    nc = bass.Bass()

    # ---- DRAM parameters (host pre-packs layouts; see _pack_weights) ----
    d_feaT = nc.declare_dram_parameter("feaT", [128, 2 * n_s], FP32, isOutput=False)
    d_ew = nc.declare_dram_parameter("elem_weights", [n_s], FP32, isOutput=False)
    d_embW = nc.declare_dram_parameter("embW", [128, 2 * 63], FP32, isOutput=False)
    d_embB = nc.declare_dram_parameter("embB", [63, 1], FP32, isOutput=False)
    d_gW1 = nc.declare_dram_parameter("gW1", [128, L * 2 * H * 2 * 128], BF16, isOutput=False)
    d_gW2m = nc.declare_dram_parameter("gW2m", [128, L * H * 2 * 64], BF16, isOutput=False)
    d_gw2g = nc.declare_dram_parameter("gw2g", [128, L * H * 2], BF16, isOutput=False)
    d_gxb = nc.declare_dram_parameter("gxb", [64, L], FP32, isOutput=False)
    d_pw = nc.declare_dram_parameter("pw", [grows, L * H], FP32, isOutput=False)
    d_b2g = nc.declare_dram_parameter("b2g", [grows, L * H], FP32, isOutput=False)
    d_cW1 = nc.declare_dram_parameter("cW1", [64, 2 * H * 2 * 128], BF16, isOutput=False)
    d_cW2m = nc.declare_dram_parameter("cW2m", [128, H * 2 * 64], BF16, isOutput=False)
    d_cw2g = nc.declare_dram_parameter("cw2g", [128, H * 2], BF16, isOutput=False)
    d_cxb = nc.declare_dram_parameter("cxb", [64, 1], FP32, isOutput=False)
    d_cpw = nc.declare_dram_parameter("cpw", [wrows, H], FP32, isOutput=False)
    d_cb2g = nc.declare_dram_parameter("cb2g", [wrows, H], FP32, isOutput=False)
    d_out = nc.declare_dram_parameter("out", [F, c_s], FP32, isOutput=True)

    with ExitStack() as ctx:
        tc = ctx.enter_context(tile.TileContext(nc))
        per = ctx.enter_context(tc.tile_pool(name="persist", bufs=1))
        dram = ctx.enter_context(tc.tile_pool(name="dram", bufs=1, space="DRAM"))
        gdram = dram.tile([H, e_s], BF16, tag="gdram", name="gdram")
        cdram = dram.tile([H, n_s], BF16, tag="cdram", name="cdram")

        # ---- persistent SBUF ----
        xT = per.tile([F, n_s], FP32, tag="xT", name="xT")
        xb = per.tile([F, n_s], BF16, tag="xb", name="xb")
        catc = per.tile([128, e_s], BF16, tag="catc", name="catc")
        embW_s = per.tile([128, 2, 63], FP32, tag="embW", name="embW")
        embB_s = per.tile([63, 1], FP32, tag="embB", name="embB")
        gW1_s = per.tile([128, L, 2, H, 2, 128], BF16, tag="gW1", name="gW1")
        gW2m_s = per.tile([128, L, H, 2, 64], BF16, tag="gW2m", name="gW2m")
        gw2g_s = per.tile([128, L, H, 2], BF16, tag="gw2g", name="gw2g")
        gxb_s = per.tile([64, L], FP32, tag="gxb", name="gxb")
        pw_s = per.tile([grows, L * H], FP32, tag="pw", name="pw")
        b2g_s = per.tile([grows, L * H], FP32, tag="b2g", name="b2g")
        cW1_s = per.tile([64, 2, H, 2, 128], BF16, tag="cW1", name="cW1")
        cW2m_s = per.tile([128, H, 2, 64], BF16, tag="cW2m", name="cW2m")
        cw2g_s = per.tile([128, H, 2], BF16, tag="cw2g", name="cw2g")
        cxb_s = per.tile([64, 1], FP32, tag="cxb", name="cxb")
        cpw_s = per.tile([wrows, H], FP32, tag="cpw", name="cpw")
        cb2g_s = per.tile([wrows, H], FP32, tag="cb2g", name="cb2g")
        lnw_s = per.tile([wrows, WCOLS], FP32, tag="lnw", name="lnw")
        lnwe_s = per.tile([grows, GCOLS], FP32, tag="lnwe", name="lnwe")
        wbuf_s = per.tile([wrows, WCOLS], FP32, tag="wbuf", name="wbuf")
        # gate logit/softmax buffers, graph layers: [125, 3, 250]
        glog = per.tile([grows, H, GCOLS], FP32, tag="glog", name="glog")
        gexp = per.tile([grows, H, GCOLS], FP32, tag="gexp", name="gexp")
        gn3 = per.tile([grows, H, GCOLS], FP32, tag="gn3", name="gn3")
        gb3 = per.tile([grows, H, GCOLS], BF16, tag="gb3", name="gb3")
        lnw3 = per.tile([grows, H, GCOLS], FP32, tag="lnw3", name="lnw3")
        ssum = per.tile([grows, H, WCOLS], FP32, tag="ssum", name="ssum")
        rb3 = per.tile([grows, H, WCOLS], FP32, tag="rb3", name="rb3")
        # pooling buffers: [125, 3, 50]
        clog = per.tile([wrows, H, WCOLS], FP32, tag="clog", name="clog")
        cexp = per.tile([wrows, H, WCOLS], FP32, tag="cexp", name="cexp")
        cn3 = per.tile([wrows, H, WCOLS], FP32, tag="cn3", name="cn3")
        cb3 = per.tile([wrows, H, WCOLS], BF16, tag="cb3", name="cb3")
        lnwc3 = per.tile([wrows, H, WCOLS], FP32, tag="lnwc3", name="lnwc3")
        csum = per.tile([wrows, H, WCOLS // K], FP32, tag="csum", name="csum")
        crb = per.tile([wrows, H, WCOLS // K], FP32, tag="crb", name="crb")
        outsum = per.tile([F, c_s], FP32, tag="outsum", name="outsum")

        # ---- load weights / constants ----
        nc.sync.dma_start(embW_s[:], d_embW[:].rearrange("p (c f) -> p c f", c=2))
        nc.sync.dma_start(embB_s[:], d_embB[:])
        nc.sync.dma_start(gW1_s[:], d_gW1[:].rearrange(
            "p (l m h c v) -> p l m h c v", l=L, m=2, h=H, c=2))
        nc.sync.dma_start(gW2m_s[:], d_gW2m[:].rearrange(
            "p (l h c f) -> p l h c f", l=L, h=H, c=2))
        nc.sync.dma_start(gw2g_s[:], d_gw2g[:].rearrange(
            "p (l h c) -> p l h c", l=L, h=H))
        nc.sync.dma_start(gxb_s[:], d_gxb[:])
        nc.sync.dma_start(pw_s[:], d_pw[:])
        nc.sync.dma_start(b2g_s[:], d_b2g[:])
        nc.sync.dma_start(cW1_s[:], d_cW1[:].rearrange(
            "p (m h c v) -> p m h c v", m=2, h=H, c=2))
        nc.sync.dma_start(cW2m_s[:], d_cW2m[:].rearrange(
            "p (h c f) -> p h c f", h=H, c=2))
        nc.sync.dma_start(cw2g_s[:], d_cw2g[:].rearrange("p (h c) -> p h c", h=H))
        nc.sync.dma_start(cxb_s[:], d_cxb[:])
        nc.sync.dma_start(cpw_s[:], d_cpw[:])
        nc.sync.dma_start(cb2g_s[:], d_cb2g[:])

        nc.sync.dma_start(xT[63:64, :], d_ew[:].unsqueeze(0))
        nc.sync.dma_start(wbuf_s[:], d_ew[:].rearrange("(r c) -> r c", r=wrows))
        nc.scalar.activation(lnw_s[:], wbuf_s[:], AF.Ln)
        # edge-expanded ln(w): lnw_e[p, c, i, j] = lnw[p, c, j]
        nc.vector.tensor_copy(
            lnwe_s[:].rearrange("p (c i j) -> p c i j", i=K, j=K),
            lnw_s[:].rearrange("p (c j) -> p c j", j=K)
            .unsqueeze(2).broadcast_to([wrows, WCOLS // K, K, K]))

        def lrelu(eng, out_ap, in_ap, tmp_pool, tshape):
            if eng == "s":
                nc.scalar.activation(out_ap, in_ap, AF.Lrelu, alpha=0.01)
            else:
                tmp = tmp_pool.tile(tshape, BF16, tag="lrt", name="lrt")
                tsz = [s for s in in_ap.shape]
                tv = tmp[tuple(slice(0, s) for s in tsz)]
                nc.vector.tensor_scalar_mul(tv, in_ap, 0.01)
                nc.vector.tensor_tensor(out_ap, in_ap, tv, op=AluOpType.max)

        # ---- embedding: xT[0:63] = (elem_fea @ embW + embB)^T, xb = bf16 ----
        with tc.tile_pool(name="emb_sb", bufs=3) as embp, \
             tc.tile_pool(name="emb_ps", bufs=2, space="PSUM") as emb_ps:
            for n0, tn in _tiles(n_s, TNE):
                stage = embp.tile([128, 2, TNE], FP32, tag="stage", name="stage")
                for c in range(2):
                    nc.sync.dma_start(
                        stage[:, c, :tn],
                        d_feaT[:, c * n_s + n0:c * n_s + n0 + tn])
                emb_o = emb_ps.tile([63, TNE], FP32, tag="emb_o", name="emb_o")
                nc.tensor.matmul(emb_o[:, :tn], embW_s[:, 0, :], stage[:, 0, :tn],
                                 start=True, stop=False)
                nc.tensor.matmul(emb_o[:, :tn], embW_s[:, 1, :], stage[:, 1, :tn],
                                 start=False, stop=True)
                nc.scalar.activation(xT[0:63, n0:n0 + tn], emb_o[:, :tn],
                                     AF.Identity, bias=embB_s[:])
                nc.vector.tensor_copy(xb[:, n0:n0 + tn], xT[:, n0:n0 + tn])

        # ---- graph message-passing layers ----
        for l in range(L):
            etiles = _tiles(e_s, TE)

            # ----- PASS 1: build catc; gate hidden -> logits -> glog -----
            with tc.tile_pool(name="p1_hg", bufs=3) as hgp, \
                 tc.tile_pool(name="p1_t", bufs=3) as tpp, \
                 tc.tile_pool(name="p1_gs", bufs=3) as gsp, \
                 tc.tile_pool(name="p1_z", bufs=2, space="PSUM") as zp, \
                 tc.tile_pool(name="p1_g", bufs=2, space="PSUM") as gp:
                for e0, te in etiles:
                    nn0, tnn = e0 // K, te // K
                    tcc = te // (K * K)
                    nc.gpsimd.tensor_copy(
                        catc[0:64, e0:e0 + te].rearrange("p (n r) -> p n r", r=K),
                        xb[:, nn0:nn0 + tnn].unsqueeze(2).broadcast_to([F, tnn, K]))
                    nc.gpsimd.tensor_copy(
                        catc[64:128, e0:e0 + te].rearrange(
                            "p (c r j) -> p c r j", r=K, j=K),
                        xb[:, nn0:nn0 + tnn].rearrange("p (c j) -> p c j", j=K)
                        .unsqueeze(2).broadcast_to([F, tcc, K, K]))
                    gt3 = gp.tile([96, 512], FP32, tag="g3", name="g3")
                    for h in range(H):
                        zt = zp.tile([128, 2, 512], FP32, tag="z", name="z")
                        hg = hgp.tile([128, 2, TE], BF16, tag="hg", name="hg")
                        for c in range(2):
                            nc.tensor.matmul(zt[:, c, :te], gW1_s[:, l, 0, h, c, :],
                                             catc[:, e0:e0 + te],
                                             start=True, stop=True)
                            lrelu(ENG_P1[h * 2 + c], hg[:, c, :te], zt[:, c, :te],
                                  tpp, [128, TE])
                        for c in range(2):
                            nc.tensor.matmul(gt3[32 * h:32 * h + 1, :te],
                                             gw2g_s[:, l, h, c:c + 1],
                                             hg[:, c, :te],
                                             start=(c == 0), stop=(c == 1))
                    r0 = e0 // GCOLS
                    for h in range(H):
                        gs = gsp.tile([1, TE], FP32, tag="gs", name="gs")
                        nc.vector.tensor_copy(gs[:, :te], gt3[32 * h:32 * h + 1, :te])
                        nc.sync.dma_start(glog[r0:r0 + te // GCOLS, h, :],
                                          gs[:, :te])

            # ----- segment softmax for all 3 heads of layer l -----
            for h in range(H):
                lh = l * H + h
                nc.vector.tensor_scalar(lnw3[:, h, :], lnwe_s[:],
                                        pw_s[:, lh:lh + 1], b2g_s[:, lh:lh + 1],
                                        op0=AluOpType.mult, op1=AluOpType.add)
            nc.vector.tensor_tensor(gexp[:], glog[:], lnw3[:], op=AluOpType.add)
            nc.scalar.activation(gexp[:], gexp[:], AF.Exp)
            nc.vector.tensor_reduce(ssum[:], gexp[:].rearrange(
                "p h (s j) -> p h s j", j=K), axis=mybir.AxisListType.X,
                op=AluOpType.add)
            nc.vector.tensor_scalar_add(ssum[:], ssum[:], 1e-10)
            nc.vector.reciprocal(rb3[:], ssum[:])
            nc.vector.tensor_tensor(
                gn3[:].rearrange("p h (s j) -> p h s j", j=K),
                gexp[:].rearrange("p h (s j) -> p h s j", j=K),
                rb3[:].unsqueeze(3).broadcast_to([grows, H, WCOLS, K]),
                op=AluOpType.mult)
            nc.vector.tensor_copy(gb3[:], gn3[:])
            for h in range(H):
                nc.sync.dma_start(gdram[h], gb3[:, h, :])

            # ----- PASS 2: msg hidden -> W2 -> gate-weighted segsum -> resid ----
            with tc.tile_pool(name="p2_hm", bufs=3) as hmp, \
                 tc.tile_pool(name="p2_t", bufs=3) as tpp, \
                 tc.tile_pool(name="p2_bc", bufs=4) as bcs, \
                 tc.tile_pool(name="p2_mw", bufs=2) as mwp, \
                 tc.tile_pool(name="p2_hs", bufs=2) as hsp, \
                 tc.tile_pool(name="p2_z", bufs=2, space="PSUM") as zp, \
                 tc.tile_pool(name="p2_w", bufs=3, space="PSUM") as wp:
                hsum_g = None
                g_nn0 = 0
                for ti, (e0, te) in enumerate(etiles):
                    nn0, tnn = e0 // K, te // K
                    if hsum_g is None:
                        hsum_g = hsp.tile([F, RES_GROUP * (TE // K)], FP32,
                                          tag="hsg", name="hsg")
                        g_nn0 = nn0
                    grow = []
                    for h in range(H):
                        bc = bcs.tile([64, TE], BF16, tag="bc", name="bc")
                        nc.sync.dma_start(
                            bc[:, :te],
                            gdram[h, e0:e0 + te].unsqueeze(0).unsqueeze(0)
                            .broadcast_to([1, 64, te]).squeeze(0))
                        grow.append(bc)
                    msgw = mwp.tile([F, TE // K, H, K], FP32, tag="mw", name="mw")
                    for h in range(H):
                        zt = zp.tile([128, 2, 512], FP32, tag="z", name="z")
                        hm = hmp.tile([128, 2, TE], BF16, tag="hm", name="hm")
                        for c in range(2):
                            nc.tensor.matmul(zt[:, c, :te], gW1_s[:, l, 1, h, c, :],
                                             catc[:, e0:e0 + te],
                                             start=True, stop=True)
                            lrelu(ENG_P2[h * 2 + c], hm[:, c, :te], zt[:, c, :te],
                                  tpp, [128, TE])
                        w2 = wp.tile([64, 512], FP32, tag="w2", name="w2")
                        nc.tensor.matmul(w2[:, :te], gW2m_s[:, l, h, 0, :],
                                         hm[:, 0, :te], start=True, stop=False)
                        nc.tensor.matmul(w2[:, :te], gW2m_s[:, l, h, 1, :],
                                         hm[:, 1, :te], start=False, stop=True)
                        nc.vector.tensor_tensor(
                            msgw[:, :tnn, h, :],
                            w2[:, :te].rearrange("p (n r) -> p n r", r=K),
                            grow[h][:, :te].rearrange("p (n r) -> p n r", r=K),
                            op=AluOpType.mult)
                    nc.vector.tensor_reduce(
                        hsum_g[:, nn0 - g_nn0:nn0 - g_nn0 + tnn],
                        msgw[:, :tnn, :, :],
                        axis=mybir.AxisListType.XY, op=AluOpType.add)
                    # residual-update chunk boundary
                    if ti % RES_GROUP == RES_GROUP - 1 or ti == len(etiles) - 1:
                        gn = nn0 + tnn - g_nn0
                        nc.vector.tensor_tensor(
                            hsum_g[:, :gn], hsum_g[:, :gn],
                            xT[:, g_nn0:g_nn0 + gn], op=AluOpType.add)
                        nc.scalar.activation(xT[:, g_nn0:g_nn0 + gn],
                                             hsum_g[:, :gn], AF.Identity,
                                             bias=gxb_s[:, l:l + 1])
                        nc.gpsimd.tensor_copy(xb[:, g_nn0:g_nn0 + gn],
                                              xT[:, g_nn0:g_nn0 + gn])
                        hsum_g = None

        # ---- crystal pooling ----
        ntiles = _tiles(n_s, TN)
        # PASS 1: gate logits
        with tc.tile_pool(name="c1_hg", bufs=3) as hgp, \
             tc.tile_pool(name="c1_t", bufs=3) as tpp, \
             tc.tile_pool(name="c1_gs", bufs=3) as gsp, \
             tc.tile_pool(name="c1_z", bufs=2, space="PSUM") as zp, \
             tc.tile_pool(name="c1_g", bufs=2, space="PSUM") as gp:
            for n0, tn in ntiles:
                gt3 = gp.tile([96, 512], FP32, tag="g3", name="g3")
                for h in range(H):
                    zt = zp.tile([128, 2, 512], FP32, tag="z", name="z")
                    hg = hgp.tile([128, 2, TN], BF16, tag="hg", name="hg")
                    for c in range(2):
                        nc.tensor.matmul(zt[:, c, :tn], cW1_s[:, 0, h, c, :],
                                         xb[:, n0:n0 + tn], start=True, stop=True)
                        lrelu(ENG_P1[h * 2 + c], hg[:, c, :tn], zt[:, c, :tn],
                              tpp, [128, TN])
                    for c in range(2):
                        nc.tensor.matmul(gt3[32 * h:32 * h + 1, :tn],
                                         cw2g_s[:, h, c:c + 1], hg[:, c, :tn],
                                         start=(c == 0), stop=(c == 1))
                r0 = n0 // WCOLS
                for h in range(H):
                    gs = gsp.tile([1, TN], FP32, tag="gs", name="gs")
                    nc.vector.tensor_copy(gs[:, :tn], gt3[32 * h:32 * h + 1, :tn])
                    nc.sync.dma_start(clog[r0:r0 + tn // WCOLS, h, :], gs[:, :tn])

        # pooling softmax (segments = 5 nodes of each crystal)
        for h in range(H):
            nc.vector.tensor_scalar(lnwc3[:, h, :], lnw_s[:],
                                    cpw_s[:, h:h + 1], cb2g_s[:, h:h + 1],
                                    op0=AluOpType.mult, op1=AluOpType.add)
        nc.vector.tensor_tensor(cexp[:], clog[:], lnwc3[:], op=AluOpType.add)
        nc.scalar.activation(cexp[:], cexp[:], AF.Exp)
        nc.vector.tensor_reduce(csum[:], cexp[:].rearrange(
            "p h (s j) -> p h s j", j=K), axis=mybir.AxisListType.X,
            op=AluOpType.add)
        nc.vector.tensor_scalar_add(csum[:], csum[:], 1e-10)
        nc.vector.reciprocal(crb[:], csum[:])
        nc.vector.tensor_tensor(
            cn3[:].rearrange("p h (s j) -> p h s j", j=K),
            cexp[:].rearrange("p h (s j) -> p h s j", j=K),
            crb[:].unsqueeze(3).broadcast_to([wrows, H, WCOLS // K, K]),
            op=AluOpType.mult)
        nc.vector.tensor_copy(cb3[:], cn3[:])
        for h in range(H):
            nc.sync.dma_start(cdram[h], cb3[:, h, :])

        # PASS 2: messages
        with tc.tile_pool(name="c2_hm", bufs=3) as hmp, \
             tc.tile_pool(name="c2_t", bufs=3) as tpp, \
             tc.tile_pool(name="c2_bc", bufs=4) as bcs, \
             tc.tile_pool(name="c2_mw", bufs=2) as mwp, \
             tc.tile_pool(name="c2_z", bufs=2, space="PSUM") as zp, \
             tc.tile_pool(name="c2_w", bufs=3, space="PSUM") as wp:
            for n0, tn in ntiles:
                cc0, tcc = n0 // K, tn // K
                grow = []
                for h in range(H):
                    bc = bcs.tile([64, TN], BF16, tag="bc", name="bc")
                    nc.sync.dma_start(
                        bc[:, :tn],
                        cdram[h, n0:n0 + tn].unsqueeze(0).unsqueeze(0)
                        .broadcast_to([1, 64, tn]).squeeze(0))
                    grow.append(bc)
                msgw = mwp.tile([F, TN // K, H, K], FP32, tag="mw", name="mw")
                for h in range(H):
                    zt = zp.tile([128, 2, 512], FP32, tag="z", name="z")
                    hm = hmp.tile([128, 2, TN], BF16, tag="hm", name="hm")
                    for c in range(2):
                        nc.tensor.matmul(zt[:, c, :tn], cW1_s[:, 1, h, c, :],
                                         xb[:, n0:n0 + tn], start=True, stop=True)
                        lrelu(ENG_P2[h * 2 + c], hm[:, c, :tn], zt[:, c, :tn],
                              tpp, [128, TN])
                    w2 = wp.tile([64, 512], FP32, tag="w2", name="w2")
                    nc.tensor.matmul(w2[:, :tn], cW2m_s[:, h, 0, :], hm[:, 0, :tn],
                                     start=True, stop=False)
                    nc.tensor.matmul(w2[:, :tn], cW2m_s[:, h, 1, :], hm[:, 1, :tn],
                                     start=False, stop=True)
                    nc.vector.tensor_tensor(
                        msgw[:, :tcc, h, :],
                        w2[:, :tn].rearrange("p (n r) -> p n r", r=K),
                        grow[h][:, :tn].rearrange("p (n r) -> p n r", r=K),
                        op=AluOpType.mult)
                nc.vector.tensor_reduce(
                    outsum[:, cc0:cc0 + tcc], msgw[:, :tcc, :, :],
                    axis=mybir.AxisListType.XY, op=AluOpType.add)

        # out = outsum + cxb, store feature-major; the host transposes
        nc.scalar.activation(outsum[:], outsum[:], AF.Identity, bias=cxb_s[:])
        nc.sync.dma_start(d_out[:], outsum[:])

    if split_waits:
        _split_multiwaits(nc)
    return nc


def _pack_weights(inp, grows, wrows):
    """Host-side packing of (replicated) weights into SBUF-ready layouts."""
    f32 = np.float32
    bf16 = ml_dtypes.bfloat16
    gW1 = np.zeros((128, L, 2, H, 2, 128), f32)
    for l in range(L):
        for h in range(H):
            for c in range(2):
                sl = slice(c * 128, (c + 1) * 128)
                gW1[:, l, 0, h, c, :] = inp["g_gate_W1"][l, h][:, sl]
                gW1[:, l, 1, h, c, :] = inp["g_msg_W1"][l, h][:, sl]
    gW2m = np.zeros((128, L, H, 2, 64), f32)
    gw2g = np.zeros((128, L, H, 2), f32)
    for l in range(L):
        for h in range(H):
            for c in range(2):
                sl = slice(c * 128, (c + 1) * 128)
                gW2m[:, l, h, c, :] = inp["g_msg_W2"][l, h][sl, :] / 3.0
                gw2g[:, l, h, c] = inp["g_gate_W2"][l, h][sl, 0]
    gxb = (np.sum(inp["g_msg_b2"], axis=1).T / 3.0).astype(f32)      # [64, L]
    pw = np.tile(np.asarray(inp["g_pow"], f32).reshape(1, L * H), (grows, 1))
    b2g = np.tile(np.asarray(inp["g_gate_b2"], f32).reshape(1, L * H), (grows, 1))

    cW1 = np.zeros((64, 2, H, 2, 128), f32)
    cW2m = np.zeros((128, H, 2, 64), f32)
    cw2g = np.zeros((128, H, 2), f32)
    for h in range(H):
        for c in range(2):
            sl = slice(c * 128, (c + 1) * 128)
            cW1[:, 0, h, c, :] = inp["c_gate_W1"][h][:, sl]
            cW1[:, 1, h, c, :] = inp["c_msg_W1"][h][:, sl]
            cW2m[:, h, c, :] = inp["c_msg_W2"][h][sl, :] / 3.0
            cw2g[:, h, c] = inp["c_gate_W2"][h][sl, 0]
    cxb = (np.sum(inp["c_msg_b2"], axis=0) / 3.0).astype(f32).reshape(64, 1)
    cpw = np.tile(np.asarray(inp["c_pow"], f32).reshape(1, H), (wrows, 1))
    cb2g = np.tile(np.asarray(inp["c_gate_b2"], f32).reshape(1, H), (wrows, 1))

    return dict(
        embW=np.pad(np.asarray(inp["emb_W"], f32), ((0, 56), (0, 0)))
        .reshape(2, 128, 63).transpose(1, 0, 2).reshape(128, 2 * 63).copy(),
        embB=np.asarray(inp["emb_b"], f32).reshape(63, 1),
        gW1=gW1.reshape(128, -1).astype(bf16),
        gW2m=gW2m.reshape(128, -1).astype(bf16),
        gw2g=gw2g.reshape(128, -1).astype(bf16),
        gxb=gxb, pw=pw, b2g=b2g,
        cW1=cW1.reshape(64, -1).astype(bf16),
        cW2m=cW2m.reshape(128, -1).astype(bf16),
        cw2g=cw2g.reshape(128, -1).astype(bf16),
        cxb=cxb, cpw=cpw, cb2g=cb2g,
    )


def prepare_in_maps(inp, c_s):
    """Build the 8 per-core input maps (weights replicated, data sharded)."""
    n_s = c_s * K
    grows = (c_s * K * K) // GCOLS
    wrows = n_s // WCOLS
    wmap = _pack_weights(inp, grows, wrows)

    fea = np.asarray(inp["elem_fea"], np.float32)
    n_tot = fea.shape[0]
    feaT = np.zeros((128, 2, n_tot), np.float32)
    ft = np.ascontiguousarray(fea.T)               # [200, N]
    feaT[:, 0, :] = ft[0:128]
    feaT[0:EMB - 128, 1, :] = ft[128:EMB]
    ew = np.asarray(inp["elem_weights"], np.float32).reshape(-1)

    in_maps = []
    for i in range(NCORES):
        m = dict(wmap)
        m["feaT"] = np.ascontiguousarray(
            feaT[:, :, i * n_s:(i + 1) * n_s]).reshape(128, 2 * n_s)
        m["elem_weights"] = ew[i * n_s:(i + 1) * n_s].copy()
        in_maps.append(m)
    return in_maps


def _check_structure(inp):
    n = inp["elem_fea"].shape[0]
    c = n // K
    e = inp["self_fea_idx"].shape[0]
    if e != c * K * K:
        return False
    if int(inp["n_crystals"]) != c:
        return False
    # all hidden biases must be zero (they are, per the spec fills)
    for k in ("g_gate_b1", "g_msg_b1", "c_gate_b1", "c_msg_b1"):
        if not np.all(np.asarray(inp[k]) == 0):
            return False
    self_ref = np.repeat(np.arange(n, dtype=np.int64), K)
    ar = np.arange(e, dtype=np.int64)
    nbr_ref = (ar // (K * K)) * K + (ar % K)
    cry_ref = np.repeat(np.arange(c, dtype=np.int64), K)
    return (np.array_equal(np.asarray(inp["self_fea_idx"]), self_ref)
            and np.array_equal(np.asarray(inp["nbr_fea_idx"]), nbr_ref)
            and np.array_equal(np.asarray(inp["cry_elem_idx"]), cry_ref))


def _reference_numpy(inp):
    """Fallback (never used when structure+zero-bias checks pass)."""
    def simple(hh, W1, b1, W2, b2):
        t = hh @ W1 + b1
        t = np.where(t > 0, t, 0.01 * t)
        return t @ W2 + b2

    def attn(fea, weights, index, nseg, gW1, gb1, gW2, gb2, mW1, mb1, mW2, mb2, p):
        gate = simple(fea, gW1, gb1, gW2, gb2)
        gmax = np.full((nseg, 1), -np.inf, np.float32)
        np.maximum.at(gmax, index[:, 0] if index.ndim > 1 else index, gate)
        gate = gate - gmax[index]
        gate = weights ** p * np.exp(gate)
        gsum = np.zeros((nseg, 1), np.float32)
        np.add.at(gsum, index, gate)
        gate = gate / (gsum[index] + 1e-10)
        msg = simple(fea, mW1, mb1, mW2, mb2)
        out = np.zeros((nseg, msg.shape[1]), np.float32)
        np.add.at(out, index, gate * msg)
        return out

    inp = {k: np.asarray(v) for k, v in inp.items()}
    n = inp["elem_fea"].shape[0]
    x = np.concatenate([inp["elem_fea"] @ inp["emb_W"] + inp["emb_b"],
                        inp["elem_weights"]], axis=1)
    w_nbr = inp["elem_weights"][inp["nbr_fea_idx"]]
    si, ni = inp["self_fea_idx"], inp["nbr_fea_idx"]
    for l in range(L):
        cat = np.concatenate([x[si], x[ni]], axis=1)
        heads = [attn(cat, w_nbr, si, n,
                      inp["g_gate_W1"][l, h], inp["g_gate_b1"][l, h],
                      inp["g_gate_W2"][l, h], inp["g_gate_b2"][l, h],
                      inp["g_msg_W1"][l, h], inp["g_msg_b1"][l, h],
                      inp["g_msg_W2"][l, h], inp["g_msg_b2"][l, h],
                      inp["g_pow"][l, h]) for h in range(H)]
        x = np.mean(heads, axis=0) + x
    ci = inp["cry_elem_idx"]
    cn = int(inp["n_crystals"])
    heads = [attn(x, inp["elem_weights"], ci, cn,
                  inp["c_gate_W1"][h], inp["c_gate_b1"][h],
                  inp["c_gate_W2"][h], inp["c_gate_b2"][h],
                  inp["c_msg_W1"][h], inp["c_msg_b1"][h],
                  inp["c_msg_W2"][h], inp["c_msg_b2"][h],
                  inp["c_pow"][h]) for h in range(H)]
    return np.mean(heads, axis=0).astype(np.float32)


_BUILT = {}


def kernel(**inputs):
    inp = {k: np.asarray(v) if not np.isscalar(v) else v for k, v in inputs.items()}
    if not _check_structure(inp):
        return _reference_numpy(inp)

    n_tot = inp["elem_fea"].shape[0]
    c_tot = n_tot // K
    assert c_tot % NCORES == 0
    c_s = c_tot // NCORES

    if c_s not in _BUILT:
        _BUILT[c_s] = build_bass(c_s)
    nc = _BUILT[c_s]

    in_maps = prepare_in_maps(inp, c_s)
    res = run_bass_kernel_spmd(nc, in_maps, list(range(NCORES)))
    out = np.concatenate(
        [np.ascontiguousarray(res.results[i]["out"].T) for i in range(NCORES)],
        axis=0)
    return out.astype(np.float32)


# revision 9
# speedup vs baseline: 1.1677x; 1.1677x over previous
"""Trainium2 Bass kernel for DescriptorNetwork (Roost-style GNN message passing).

Structure exploited (verified at runtime in kernel()):
  - N = C*K nodes, K=5 elements per crystal, edges = all-pairs within crystal
  - self_fea_idx = repeat(arange(N), 5), nbr_fea_idx = per-crystal tile,
    cry_elem_idx = repeat(arange(C), 5)
  => every gather is a strided/broadcast access pattern; every segment
     reduction is over 5 contiguous elements.

Sharding: 1250 crystals per core x 8 cores, fully data parallel, no
collectives.  Everything on-chip is feature-major (features on SBUF
partitions, nodes/edges along the free dimension).

v2 layout (vs the first working version):
  - elem_fea is transposed host-side -> no on-chip transposes for embedding
  - x keeps an fp32 master (xT) plus a bf16 mirror (xb); catT is built once
    per layer into a persistent bf16 cache and reused by both passes
  - Lrelu activations are split between the scalar engine (ACT) and the
    vector engine (mul+max 2-op form; biases are all zero per the spec)
  - gate broadcast to 64 partitions via a K=1 ones-matmul on the PE from a
    bf16 copy of the normalized gate (no DRAM bounce)
  - pooling matmuls run in bf16 from xb
  - output stays feature-major [64, c_s]; the host transposes
  - residual updates are chunked so the next layer starts while the tail of
    the current layer finishes (keeps the PE HAM clock warm)
"""

import numpy as np
import ml_dtypes
from contextlib import ExitStack

import concourse.bass as bass
import concourse.tile as tile
from concourse import mybir
from concourse.alu_op_type import AluOpType
from concourse.bass_utils import run_bass_kernel_spmd

FP32 = mybir.dt.float32
BF16 = mybir.dt.bfloat16
AF = mybir.ActivationFunctionType

# Model constants (hardcoded per problem spec)
C_TOT = 10000
K = 5
N_TOT = C_TOT * K
EMB = 200
F = 64
L = 3
H = 3
HID = 256
NCORES = 8

C_S = C_TOT // NCORES          # 1250 crystals per core
GCOLS = 250                    # gate buffer: 250 edges (10 crystals) per row
WCOLS = 50                     # node buffer: 50 nodes (10 crystals) per row
TE = 500                       # edge tile (100 nodes, 20 crystals)
TN = 500                       # node tile for pooling
TNE = 512                      # node tile for embedding
RES_GROUP = 16                 # pass-2 tiles per residual-update chunk

# Lrelu engine per (head, chunk) slot: 's'=scalar ACT, 'v'=vector 2-op
ENG_P1 = ["s", "v", "s", "s", "v", "s"]
ENG_P2 = ["s", "v", "s", "s", "v", "s"]


def _tiles(total, size):
    out, o = [], 0
    while o < total:
        out.append((o, min(size, total - o)))
        o += size
    return out


def _split_multiwaits(nc):
    """Walrus in this container encodes at most one on_wait per instruction;
    Tile emits several.  Split extras into preceding wait-only instructions."""
    n_split = 0
    for bb in nc.main_func.blocks:
        new = []
        for inst in bb.instructions:
            si = getattr(inst, "sync_info", None)
            waits = list(si.on_wait) if (si is not None and si.on_wait) else []
            if len(waits) > 1:
                for w in waits[:-1]:
                    ev = mybir.InstEventSemaphore(
                        name=f"{inst.name}-w{n_split}",
                        ins=[], outs=[],
                        sync_info=mybir.SyncInfo(on_wait=[w], on_update=[]),
                    )
                    ev.engine = inst.engine
                    new.append(ev)
                    n_split += 1
                si.on_wait = [waits[-1]]
            new.append(inst)
        bb.instructions[:] = new
    return n_split


def build_bass(c_s=C_S, split_waits=True):
    """Build the per-core Bass program (same program on all 8 cores).
    Assumes all hidden-layer biases (b1) are zero (guaranteed by the spec
    fills; checked host-side with fallback)."""
    n_s, e_s = c_s * K, c_s * K * K
    assert e_s % GCOLS == 0 and n_s % WCOLS == 0
    grows, wrows = e_s // GCOLS, n_s // WCOLS

    nc = bass.Bass()

    # ---- DRAM parameters (host pre-packs layouts; see _pack_weights) ----
    d_feaT = nc.declare_dram_parameter("feaT", [128, 2 * n_s], FP32, isOutput=False)
    d_ew = nc.declare_dram_parameter("elem_weights", [n_s], FP32, isOutput=False)
    d_embW = nc.declare_dram_parameter("embW", [128, 2 * 63], FP32, isOutput=False)
    d_embB = nc.declare_dram_parameter("embB", [63, 1], FP32, isOutput=False)
    d_gW1 = nc.declare_dram_parameter("gW1", [128, L * 2 * H * 2 * 128], BF16, isOutput=False)
    d_gW2m = nc.declare_dram_parameter("gW2m", [128, L * H * 2 * 64], BF16, isOutput=False)
    d_gw2g = nc.declare_dram_parameter("gw2g", [128, L * H * 2], BF16, isOutput=False)
    d_gxb = nc.declare_dram_parameter("gxb", [64, L], FP32, isOutput=False)
    d_pw = nc.declare_dram_parameter("pw", [grows, L * H], FP32, isOutput=False)
    d_b2g = nc.declare_dram_parameter("b2g", [grows, L * H], FP32, isOutput=False)
    d_cW1 = nc.declare_dram_parameter("cW1", [64, 2 * H * 2 * 128], BF16, isOutput=False)
    d_cW2m = nc.declare_dram_parameter("cW2m", [128, H * 2 * 64], BF16, isOutput=False)
    d_cw2g = nc.declare_dram_parameter("cw2g", [128, H * 2], BF16, isOutput=False)
    d_cxb = nc.declare_dram_parameter("cxb", [64, 1], FP32, isOutput=False)
    d_cpw = nc.declare_dram_parameter("cpw", [wrows, H], FP32, isOutput=False)
    d_cb2g = nc.declare_dram_parameter("cb2g", [wrows, H], FP32, isOutput=False)
    d_out = nc.declare_dram_parameter("out", [F, c_s], FP32, isOutput=True)

    with ExitStack() as ctx:
        tc = ctx.enter_context(tile.TileContext(nc))
        per = ctx.enter_context(tc.tile_pool(name="persist", bufs=1))
        dram = ctx.enter_context(tc.tile_pool(name="dram", bufs=1, space="DRAM"))
        gdram = dram.tile([H, e_s], BF16, tag="gdram", name="gdram")
        cdram = dram.tile([H, n_s], BF16, tag="cdram", name="cdram")

        # ---- persistent SBUF ----
        xT = per.tile([F, n_s], FP32, tag="xT", name="xT")
        xb = per.tile([F, n_s], BF16, tag="xb", name="xb")
        catc = per.tile([128, e_s], BF16, tag="catc", name="catc")
        embW_s = per.tile([128, 2, 63], FP32, tag="embW", name="embW")
        embB_s = per.tile([63, 1], FP32, tag="embB", name="embB")
        gW1_s = per.tile([128, L, 2, H, 2, 128], BF16, tag="gW1", name="gW1")
        gW2m_s = per.tile([128, L, H, 2, 64], BF16, tag="gW2m", name="gW2m")
        gw2g_s = per.tile([128, L, H, 2], BF16, tag="gw2g", name="gw2g")
        gxb_s = per.tile([64, L], FP32, tag="gxb", name="gxb")
        pw_s = per.tile([grows, L * H], FP32, tag="pw", name="pw")
        b2g_s = per.tile([grows, L * H], FP32, tag="b2g", name="b2g")
        cW1_s = per.tile([64, 2, H, 2, 128], BF16, tag="cW1", name="cW1")
        cW2m_s = per.tile([128, H, 2, 64], BF16, tag="cW2m", name="cW2m")
        cw2g_s = per.tile([128, H, 2], BF16, tag="cw2g", name="cw2g")
        cxb_s = per.tile([64, 1], FP32, tag="cxb", name="cxb")
        cpw_s = per.tile([wrows, H], FP32, tag="cpw", name="cpw")
        cb2g_s = per.tile([wrows, H], FP32, tag="cb2g", name="cb2g")
        lnw_s = per.tile([wrows, WCOLS], FP32, tag="lnw", name="lnw")
        lnwe_s = per.tile([grows, GCOLS], FP32, tag="lnwe", name="lnwe")
        wbuf_s = per.tile([wrows, WCOLS], FP32, tag="wbuf", name="wbuf")
        # gate logit/softmax buffers, graph layers: [125, 3, 250]
        glog = per.tile([grows, H, GCOLS], FP32, tag="glog", name="glog")
        gexp = per.tile([grows, H, GCOLS], FP32, tag="gexp", name="gexp")
        gn3 = per.tile([grows, H, GCOLS], FP32, tag="gn3", name="gn3")
        gb3 = per.tile([grows, H, GCOLS], BF16, tag="gb3", name="gb3")
        lnw3 = per.tile([grows, H, GCOLS], FP32, tag="lnw3", name="lnw3")
        ssum = per.tile([grows, H, WCOLS], FP32, tag="ssum", name="ssum")
        rb3 = per.tile([grows, H, WCOLS], FP32, tag="rb3", name="rb3")
        # pooling buffers: [125, 3, 50]
        clog = per.tile([wrows, H, WCOLS], FP32, tag="clog", name="clog")
        cexp = per.tile([wrows, H, WCOLS], FP32, tag="cexp", name="cexp")
        cn3 = per.tile([wrows, H, WCOLS], FP32, tag="cn3", name="cn3")
        cb3 = per.tile([wrows, H, WCOLS], BF16, tag="cb3", name="cb3")
        lnwc3 = per.tile([wrows, H, WCOLS], FP32, tag="lnwc3", name="lnwc3")
        csum = per.tile([wrows, H, WCOLS // K], FP32, tag="csum", name="csum")
        crb = per.tile([wrows, H, WCOLS // K], FP32, tag="crb", name="crb")
        outsum = per.tile([F, c_s], FP32, tag="outsum", name="outsum")

        # ---- load weights / constants ----
        nc.sync.dma_start(embW_s[:], d_embW[:].rearrange("p (c f) -> p c f", c=2))
        nc.sync.dma_start(embB_s[:], d_embB[:])
        nc.sync.dma_start(gW1_s[:], d_gW1[:].rearrange(
            "p (l m h c v) -> p l m h c v", l=L, m=2, h=H, c=2))
        nc.sync.dma_start(gW2m_s[:], d_gW2m[:].rearrange(
            "p (l h c f) -> p l h c f", l=L, h=H, c=2))
        nc.sync.dma_start(gw2g_s[:], d_gw2g[:].rearrange(
            "p (l h c) -> p l h c", l=L, h=H))
        nc.sync.dma_start(gxb_s[:], d_gxb[:])
        nc.sync.dma_start(pw_s[:], d_pw[:])
        nc.sync.dma_start(b2g_s[:], d_b2g[:])
        nc.sync.dma_start(cW1_s[:], d_cW1[:].rearrange(
            "p (m h c v) -> p m h c v", m=2, h=H, c=2))
        nc.sync.dma_start(cW2m_s[:], d_cW2m[:].rearrange(
            "p (h c f) -> p h c f", h=H, c=2))
        nc.sync.dma_start(cw2g_s[:], d_cw2g[:].rearrange("p (h c) -> p h c", h=H))
        nc.sync.dma_start(cxb_s[:], d_cxb[:])
        nc.sync.dma_start(cpw_s[:], d_cpw[:])
        nc.sync.dma_start(cb2g_s[:], d_cb2g[:])

        nc.sync.dma_start(xT[63:64, :], d_ew[:].unsqueeze(0))
        nc.sync.dma_start(wbuf_s[:], d_ew[:].rearrange("(r c) -> r c", r=wrows))
        nc.scalar.activation(lnw_s[:], wbuf_s[:], AF.Ln)
        # edge-expanded ln(w): lnw_e[p, c, i, j] = lnw[p, c, j]
        nc.vector.tensor_copy(
            lnwe_s[:].rearrange("p (c i j) -> p c i j", i=K, j=K),
            lnw_s[:].rearrange("p (c j) -> p c j", j=K)
            .unsqueeze(2).broadcast_to([wrows, WCOLS // K, K, K]))

        def lrelu(eng, out_ap, in_ap, tmp_pool, tshape):
            if eng == "s":
                nc.scalar.activation(out_ap, in_ap, AF.Lrelu, alpha=0.01)
            else:
                tmp = tmp_pool.tile(tshape, BF16, tag="lrt", name="lrt")
                tsz = [s for s in in_ap.shape]
                tv = tmp[tuple(slice(0, s) for s in tsz)]
                nc.vector.tensor_scalar_mul(tv, in_ap, 0.01)
                nc.vector.tensor_tensor(out_ap, in_ap, tv, op=AluOpType.max)

        # ---- embedding: xT[0:63] = (elem_fea @ embW + embB)^T, xb = bf16 ----
        with tc.tile_pool(name="emb_sb", bufs=3) as embp, \
             tc.tile_pool(name="emb_ps", bufs=2, space="PSUM") as emb_ps:
            for n0, tn in _tiles(n_s, TNE):
                stage = embp.tile([128, 2, TNE], FP32, tag="stage", name="stage")
                for c in range(2):
                    nc.sync.dma_start(
                        stage[:, c, :tn],
                        d_feaT[:, c * n_s + n0:c * n_s + n0 + tn])
                emb_o = emb_ps.tile([63, TNE], FP32, tag="emb_o", name="emb_o")
                nc.tensor.matmul(emb_o[:, :tn], embW_s[:, 0, :], stage[:, 0, :tn],
                                 start=True, stop=False)
                nc.tensor.matmul(emb_o[:, :tn], embW_s[:, 1, :], stage[:, 1, :tn],
                                 start=False, stop=True)
                nc.scalar.activation(xT[0:63, n0:n0 + tn], emb_o[:, :tn],
                                     AF.Identity, bias=embB_s[:])
                nc.vector.tensor_copy(xb[:, n0:n0 + tn], xT[:, n0:n0 + tn])

        # ---- graph message-passing layers ----
        for l in range(L):
            etiles = _tiles(e_s, TE)

            # ----- PASS 1: build catc; gate hidden -> logits -> glog -----
            with tc.tile_pool(name="p1_hg", bufs=3) as hgp, \
                 tc.tile_pool(name="p1_t", bufs=3) as tpp, \
                 tc.tile_pool(name="p1_gs", bufs=3) as gsp, \
                 tc.tile_pool(name="p1_z", bufs=2, space="PSUM") as zp, \
                 tc.tile_pool(name="p1_g", bufs=2, space="PSUM") as gp:
                for e0, te in etiles:
                    nn0, tnn = e0 // K, te // K
                    tcc = te // (K * K)
                    nc.gpsimd.tensor_copy(
                        catc[0:64, e0:e0 + te].rearrange("p (n r) -> p n r", r=K),
                        xb[:, nn0:nn0 + tnn].unsqueeze(2).broadcast_to([F, tnn, K]))
                    nc.gpsimd.tensor_copy(
                        catc[64:128, e0:e0 + te].rearrange(
                            "p (c r j) -> p c r j", r=K, j=K),
                        xb[:, nn0:nn0 + tnn].rearrange("p (c j) -> p c j", j=K)
                        .unsqueeze(2).broadcast_to([F, tcc, K, K]))
                    gt3 = gp.tile([96, 512], FP32, tag="g3", name="g3")
                    for h in range(H):
                        zt = zp.tile([128, 2, 512], FP32, tag="z", name="z")
                        hg = hgp.tile([128, 2, TE], BF16, tag="hg", name="hg")
                        for c in range(2):
                            nc.tensor.matmul(zt[:, c, :te], gW1_s[:, l, 0, h, c, :],
                                             catc[:, e0:e0 + te],
                                             start=True, stop=True)
                        eng = "v" if ((e0 // TE) * H + h) % 4 == 3 else "s"
                        lrelu(eng, hg[:, :, :te], zt[:, :, :te], tpp, [128, 2, TE])
                        for c in range(2):
                            nc.tensor.matmul(gt3[32 * h:32 * h + 1, :te],
                                             gw2g_s[:, l, h, c:c + 1],
                                             hg[:, c, :te],
                                             start=(c == 0), stop=(c == 1))
                    r0 = e0 // GCOLS
                    gs3 = gsp.tile([96, TE], FP32, tag="gs", name="gs")
                    nc.vector.tensor_copy(gs3[:, :te], gt3[0:96, :te])
                    for h in range(H):
                        nc.sync.dma_start(glog[r0:r0 + te // GCOLS, h, :],
                                          gs3[32 * h:32 * h + 1, :te])

            # ----- segment softmax for all 3 heads of layer l -----
            for h in range(H):
                lh = l * H + h
                nc.vector.tensor_scalar(lnw3[:, h, :], lnwe_s[:],
                                        pw_s[:, lh:lh + 1], b2g_s[:, lh:lh + 1],
                                        op0=AluOpType.mult, op1=AluOpType.add)
            nc.gpsimd.tensor_tensor(gexp[:], glog[:], lnw3[:], op=AluOpType.add)
            nc.scalar.activation(gexp[:], gexp[:], AF.Exp)
            nc.vector.tensor_reduce(ssum[:], gexp[:].rearrange(
                "p h (s j) -> p h s j", j=K), axis=mybir.AxisListType.X,
                op=AluOpType.add)
            nc.vector.tensor_scalar_add(ssum[:], ssum[:], 1e-10)
            nc.vector.reciprocal(rb3[:], ssum[:])
            nc.vector.tensor_tensor(
                gn3[:].rearrange("p h (s j) -> p h s j", j=K),
                gexp[:].rearrange("p h (s j) -> p h s j", j=K),
                rb3[:].unsqueeze(3).broadcast_to([grows, H, WCOLS, K]),
                op=AluOpType.mult)
            nc.gpsimd.tensor_copy(gb3[:], gn3[:])
            for h in range(H):
                nc.sync.dma_start(gdram[h], gb3[:, h, :])

            # ----- PASS 2: msg hidden -> W2 -> gate-weighted segsum -> resid ----
            with tc.tile_pool(name="p2_hm", bufs=3) as hmp, \
                 tc.tile_pool(name="p2_t", bufs=3) as tpp, \
                 tc.tile_pool(name="p2_bc", bufs=4) as bcs, \
                 tc.tile_pool(name="p2_mw", bufs=2) as mwp, \
                 tc.tile_pool(name="p2_hs", bufs=2) as hsp, \
                 tc.tile_pool(name="p2_z", bufs=2, space="PSUM") as zp, \
                 tc.tile_pool(name="p2_w", bufs=3, space="PSUM") as wp:
                hsum_g = None
                g_nn0 = 0
                for ti, (e0, te) in enumerate(etiles):
                    nn0, tnn = e0 // K, te // K
                    if hsum_g is None:
                        hsum_g = hsp.tile([F, RES_GROUP * (TE // K)], FP32,
                                          tag="hsg", name="hsg")
                        g_nn0 = nn0
                    grow = []
                    for h in range(H):
                        bc = bcs.tile([64, TE], BF16, tag="bc", name="bc")
                        nc.sync.dma_start(
                            bc[:, :te],
                            gdram[h, e0:e0 + te].unsqueeze(0).unsqueeze(0)
                            .broadcast_to([1, 64, te]).squeeze(0))
                        grow.append(bc)
                    msgw = mwp.tile([F, TE // K, H, K], BF16, tag="mw", name="mw")
                    for h in range(H):
                        zt = zp.tile([128, 2, 512], FP32, tag="z", name="z")
                        hm = hmp.tile([128, 2, TE], BF16, tag="hm", name="hm")
                        for c in range(2):
                            nc.tensor.matmul(zt[:, c, :te], gW1_s[:, l, 1, h, c, :],
                                             catc[:, e0:e0 + te],
                                             start=True, stop=True)
                        eng = "v" if (ti * H + h) % 4 == 1 else "s"
                        lrelu(eng, hm[:, :, :te], zt[:, :, :te], tpp, [128, 2, TE])
                        w2 = wp.tile([64, 512], FP32, tag="w2", name="w2")
                        nc.tensor.matmul(w2[:, :te], gW2m_s[:, l, h, 0, :],
                                         hm[:, 0, :te], start=True, stop=False)
                        nc.tensor.matmul(w2[:, :te], gW2m_s[:, l, h, 1, :],
                                         hm[:, 1, :te], start=False, stop=True)
                        nc.vector.tensor_tensor(
                            msgw[:, :tnn, h, :],
                            w2[:, :te].rearrange("p (n r) -> p n r", r=K),
                            grow[h][:, :te].rearrange("p (n r) -> p n r", r=K),
                            op=AluOpType.mult)
                    nc.vector.tensor_reduce(
                        hsum_g[:, nn0 - g_nn0:nn0 - g_nn0 + tnn],
                        msgw[:, :tnn, :, :],
                        axis=mybir.AxisListType.XY, op=AluOpType.add)
                    # residual-update chunk boundary
                    if ti % RES_GROUP == RES_GROUP - 1 or ti == len(etiles) - 1:
                        gn = nn0 + tnn - g_nn0
                        nc.gpsimd.tensor_tensor(
                            hsum_g[:, :gn], hsum_g[:, :gn],
                            xT[:, g_nn0:g_nn0 + gn], op=AluOpType.add)
                        nc.scalar.activation(xT[:, g_nn0:g_nn0 + gn],
                                             hsum_g[:, :gn], AF.Identity,
                                             bias=gxb_s[:, l:l + 1])
                        nc.gpsimd.tensor_copy(xb[:, g_nn0:g_nn0 + gn],
                                              xT[:, g_nn0:g_nn0 + gn])
                        hsum_g = None

        # ---- crystal pooling ----
        ntiles = _tiles(n_s, TN)
        # PASS 1: gate logits
        with tc.tile_pool(name="c1_hg", bufs=3) as hgp, \
             tc.tile_pool(name="c1_t", bufs=3) as tpp, \
             tc.tile_pool(name="c1_gs", bufs=3) as gsp, \
             tc.tile_pool(name="c1_z", bufs=2, space="PSUM") as zp, \
             tc.tile_pool(name="c1_g", bufs=2, space="PSUM") as gp:
            for n0, tn in ntiles:
                gt3 = gp.tile([96, 512], FP32, tag="g3", name="g3")
                for h in range(H):
                    zt = zp.tile([128, 2, 512], FP32, tag="z", name="z")
                    hg = hgp.tile([128, 2, TN], BF16, tag="hg", name="hg")
                    for c in range(2):
                        nc.tensor.matmul(zt[:, c, :tn], cW1_s[:, 0, h, c, :],
                                         xb[:, n0:n0 + tn], start=True, stop=True)
                    eng = "v" if ((n0 // TN) * H + h) % 4 == 3 else "s"
                    lrelu(eng, hg[:, :, :tn], zt[:, :, :tn], tpp, [128, 2, TN])
                    for c in range(2):
                        nc.tensor.matmul(gt3[32 * h:32 * h + 1, :tn],
                                         cw2g_s[:, h, c:c + 1], hg[:, c, :tn],
                                         start=(c == 0), stop=(c == 1))
                r0 = n0 // WCOLS
                gs3 = gsp.tile([96, TN], FP32, tag="gs", name="gs")
                nc.vector.tensor_copy(gs3[:, :tn], gt3[0:96, :tn])
                for h in range(H):
                    nc.sync.dma_start(clog[r0:r0 + tn // WCOLS, h, :],
                                      gs3[32 * h:32 * h + 1, :tn])

        # pooling softmax (segments = 5 nodes of each crystal)
        for h in range(H):
            nc.vector.tensor_scalar(lnwc3[:, h, :], lnw_s[:],
                                    cpw_s[:, h:h + 1], cb2g_s[:, h:h + 1],
                                    op0=AluOpType.mult, op1=AluOpType.add)
        nc.gpsimd.tensor_tensor(cexp[:], clog[:], lnwc3[:], op=AluOpType.add)
        nc.scalar.activation(cexp[:], cexp[:], AF.Exp)
        nc.vector.tensor_reduce(csum[:], cexp[:].rearrange(
            "p h (s j) -> p h s j", j=K), axis=mybir.AxisListType.X,
            op=AluOpType.add)
        nc.vector.tensor_scalar_add(csum[:], csum[:], 1e-10)
        nc.vector.reciprocal(crb[:], csum[:])
        nc.vector.tensor_tensor(
            cn3[:].rearrange("p h (s j) -> p h s j", j=K),
            cexp[:].rearrange("p h (s j) -> p h s j", j=K),
            crb[:].unsqueeze(3).broadcast_to([wrows, H, WCOLS // K, K]),
            op=AluOpType.mult)
        nc.gpsimd.tensor_copy(cb3[:], cn3[:])
        for h in range(H):
            nc.sync.dma_start(cdram[h], cb3[:, h, :])

        # PASS 2: messages
        with tc.tile_pool(name="c2_hm", bufs=3) as hmp, \
             tc.tile_pool(name="c2_t", bufs=3) as tpp, \
             tc.tile_pool(name="c2_bc", bufs=4) as bcs, \
             tc.tile_pool(name="c2_mw", bufs=2) as mwp, \
             tc.tile_pool(name="c2_z", bufs=2, space="PSUM") as zp, \
             tc.tile_pool(name="c2_w", bufs=3, space="PSUM") as wp:
            for n0, tn in ntiles:
                cc0, tcc = n0 // K, tn // K
                grow = []
                for h in range(H):
                    bc = bcs.tile([64, TN], BF16, tag="bc", name="bc")
                    nc.sync.dma_start(
                        bc[:, :tn],
                        cdram[h, n0:n0 + tn].unsqueeze(0).unsqueeze(0)
                        .broadcast_to([1, 64, tn]).squeeze(0))
                    grow.append(bc)
                msgw = mwp.tile([F, TN // K, H, K], BF16, tag="mw", name="mw")
                for h in range(H):
                    zt = zp.tile([128, 2, 512], FP32, tag="z", name="z")
                    hm = hmp.tile([128, 2, TN], BF16, tag="hm", name="hm")
                    for c in range(2):
                        nc.tensor.matmul(zt[:, c, :tn], cW1_s[:, 1, h, c, :],
                                         xb[:, n0:n0 + tn], start=True, stop=True)
                    eng = "v" if ((n0 // TN) * H + h) % 4 == 1 else "s"
                    lrelu(eng, hm[:, :, :tn], zt[:, :, :tn], tpp, [128, 2, TN])
                    w2 = wp.tile([64, 512], FP32, tag="w2", name="w2")
                    nc.tensor.matmul(w2[:, :tn], cW2m_s[:, h, 0, :], hm[:, 0, :tn],
                                     start=True, stop=False)
                    nc.tensor.matmul(w2[:, :tn], cW2m_s[:, h, 1, :], hm[:, 1, :tn],
                                     start=False, stop=True)
                    nc.vector.tensor_tensor(
                        msgw[:, :tcc, h, :],
                        w2[:, :tn].rearrange("p (n r) -> p n r", r=K),
                        grow[h][:, :tn].rearrange("p (n r) -> p n r", r=K),
                        op=AluOpType.mult)
                nc.vector.tensor_reduce(
                    outsum[:, cc0:cc0 + tcc], msgw[:, :tcc, :, :],
                    axis=mybir.AxisListType.XY, op=AluOpType.add)

        # out = outsum + cxb, store feature-major; the host transposes
        nc.scalar.activation(outsum[:], outsum[:], AF.Identity, bias=cxb_s[:])
        nc.sync.dma_start(d_out[:], outsum[:])

    if split_waits:
        _split_multiwaits(nc)
    return nc


def _pack_weights(inp, grows, wrows):
    """Host-side packing of (replicated) weights into SBUF-ready layouts."""
    f32 = np.float32
    bf16 = ml_dtypes.bfloat16
    gW1 = np.zeros((128, L, 2, H, 2, 128), f32)
    for l in range(L):
        for h in range(H):
            for c in range(2):
                sl = slice(c * 128, (c + 1) * 128)
                gW1[:, l, 0, h, c, :] = inp["g_gate_W1"][l, h][:, sl]
                gW1[:, l, 1, h, c, :] = inp["g_msg_W1"][l, h][:, sl]
    gW2m = np.zeros((128, L, H, 2, 64), f32)
    gw2g = np.zeros((128, L, H, 2), f32)
    for l in range(L):
        for h in range(H):
            for c in range(2):
                sl = slice(c * 128, (c + 1) * 128)
                gW2m[:, l, h, c, :] = inp["g_msg_W2"][l, h][sl, :] / 3.0
                gw2g[:, l, h, c] = inp["g_gate_W2"][l, h][sl, 0]
    gxb = (np.sum(inp["g_msg_b2"], axis=1).T / 3.0).astype(f32)      # [64, L]
    pw = np.tile(np.asarray(inp["g_pow"], f32).reshape(1, L * H), (grows, 1))
    b2g = np.tile(np.asarray(inp["g_gate_b2"], f32).reshape(1, L * H), (grows, 1))

    cW1 = np.zeros((64, 2, H, 2, 128), f32)
    cW2m = np.zeros((128, H, 2, 64), f32)
    cw2g = np.zeros((128, H, 2), f32)
    for h in range(H):
        for c in range(2):
            sl = slice(c * 128, (c + 1) * 128)
            cW1[:, 0, h, c, :] = inp["c_gate_W1"][h][:, sl]
            cW1[:, 1, h, c, :] = inp["c_msg_W1"][h][:, sl]
            cW2m[:, h, c, :] = inp["c_msg_W2"][h][sl, :] / 3.0
            cw2g[:, h, c] = inp["c_gate_W2"][h][sl, 0]
    cxb = (np.sum(inp["c_msg_b2"], axis=0) / 3.0).astype(f32).reshape(64, 1)
    cpw = np.tile(np.asarray(inp["c_pow"], f32).reshape(1, H), (wrows, 1))
    cb2g = np.tile(np.asarray(inp["c_gate_b2"], f32).reshape(1, H), (wrows, 1))

    return dict(
        embW=np.pad(np.asarray(inp["emb_W"], f32), ((0, 56), (0, 0)))
        .reshape(2, 128, 63).transpose(1, 0, 2).reshape(128, 2 * 63).copy(),
        embB=np.asarray(inp["emb_b"], f32).reshape(63, 1),
        gW1=gW1.reshape(128, -1).astype(bf16),
        gW2m=gW2m.reshape(128, -1).astype(bf16),
        gw2g=gw2g.reshape(128, -1).astype(bf16),
        gxb=gxb, pw=pw, b2g=b2g,
        cW1=cW1.reshape(64, -1).astype(bf16),
        cW2m=cW2m.reshape(128, -1).astype(bf16),
        cw2g=cw2g.reshape(128, -1).astype(bf16),
        cxb=cxb, cpw=cpw, cb2g=cb2g,
    )


def prepare_in_maps(inp, c_s):
    """Build the 8 per-core input maps (weights replicated, data sharded)."""
    n_s = c_s * K
    grows = (c_s * K * K) // GCOLS
    wrows = n_s // WCOLS
    wmap = _pack_weights(inp, grows, wrows)

    fea = np.asarray(inp["elem_fea"], np.float32)
    n_tot = fea.shape[0]
    feaT = np.zeros((128, 2, n_tot), np.float32)
    ft = np.ascontiguousarray(fea.T)               # [200, N]
    feaT[:, 0, :] = ft[0:128]
    feaT[0:EMB - 128, 1, :] = ft[128:EMB]
    ew = np.asarray(inp["elem_weights"], np.float32).reshape(-1)

    in_maps = []
    for i in range(NCORES):
        m = dict(wmap)
        m["feaT"] = np.ascontiguousarray(
            feaT[:, :, i * n_s:(i + 1) * n_s]).reshape(128, 2 * n_s)
        m["elem_weights"] = ew[i * n_s:(i + 1) * n_s].copy()
        in_maps.append(m)
    return in_maps


def _check_structure(inp):
    n = inp["elem_fea"].shape[0]
    c = n // K
    e = inp["self_fea_idx"].shape[0]
    if e != c * K * K:
        return False
    if int(inp["n_crystals"]) != c:
        return False
    # all hidden biases must be zero (they are, per the spec fills)
    for k in ("g_gate_b1", "g_msg_b1", "c_gate_b1", "c_msg_b1"):
        if not np.all(np.asarray(inp[k]) == 0):
            return False
    self_ref = np.repeat(np.arange(n, dtype=np.int64), K)
    ar = np.arange(e, dtype=np.int64)
    nbr_ref = (ar // (K * K)) * K + (ar % K)
    cry_ref = np.repeat(np.arange(c, dtype=np.int64), K)
    return (np.array_equal(np.asarray(inp["self_fea_idx"]), self_ref)
            and np.array_equal(np.asarray(inp["nbr_fea_idx"]), nbr_ref)
            and np.array_equal(np.asarray(inp["cry_elem_idx"]), cry_ref))


def _reference_numpy(inp):
    """Fallback (never used when structure+zero-bias checks pass)."""
    def simple(hh, W1, b1, W2, b2):
        t = hh @ W1 + b1
        t = np.where(t > 0, t, 0.01 * t)
        return t @ W2 + b2

    def attn(fea, weights, index, nseg, gW1, gb1, gW2, gb2, mW1, mb1, mW2, mb2, p):
        gate = simple(fea, gW1, gb1, gW2, gb2)
        gmax = np.full((nseg, 1), -np.inf, np.float32)
        np.maximum.at(gmax, index[:, 0] if index.ndim > 1 else index, gate)
        gate = gate - gmax[index]
        gate = weights ** p * np.exp(gate)
        gsum = np.zeros((nseg, 1), np.float32)
        np.add.at(gsum, index, gate)
        gate = gate / (gsum[index] + 1e-10)
        msg = simple(fea, mW1, mb1, mW2, mb2)
        out = np.zeros((nseg, msg.shape[1]), np.float32)
        np.add.at(out, index, gate * msg)
        return out

    inp = {k: np.asarray(v) for k, v in inp.items()}
    n = inp["elem_fea"].shape[0]
    x = np.concatenate([inp["elem_fea"] @ inp["emb_W"] + inp["emb_b"],
                        inp["elem_weights"]], axis=1)
    w_nbr = inp["elem_weights"][inp["nbr_fea_idx"]]
    si, ni = inp["self_fea_idx"], inp["nbr_fea_idx"]
    for l in range(L):
        cat = np.concatenate([x[si], x[ni]], axis=1)
        heads = [attn(cat, w_nbr, si, n,
                      inp["g_gate_W1"][l, h], inp["g_gate_b1"][l, h],
                      inp["g_gate_W2"][l, h], inp["g_gate_b2"][l, h],
                      inp["g_msg_W1"][l, h], inp["g_msg_b1"][l, h],
                      inp["g_msg_W2"][l, h], inp["g_msg_b2"][l, h],
                      inp["g_pow"][l, h]) for h in range(H)]
        x = np.mean(heads, axis=0) + x
    ci = inp["cry_elem_idx"]
    cn = int(inp["n_crystals"])
    heads = [attn(x, inp["elem_weights"], ci, cn,
                  inp["c_gate_W1"][h], inp["c_gate_b1"][h],
                  inp["c_gate_W2"][h], inp["c_gate_b2"][h],
                  inp["c_msg_W1"][h], inp["c_msg_b1"][h],
                  inp["c_msg_W2"][h], inp["c_msg_b2"][h],
                  inp["c_pow"][h]) for h in range(H)]
    return np.mean(heads, axis=0).astype(np.float32)


_BUILT = {}


def kernel(**inputs):
    inp = {k: np.asarray(v) if not np.isscalar(v) else v for k, v in inputs.items()}
    if not _check_structure(inp):
        return _reference_numpy(inp)

    n_tot = inp["elem_fea"].shape[0]
    c_tot = n_tot // K
    assert c_tot % NCORES == 0
    c_s = c_tot // NCORES

    if c_s not in _BUILT:
        _BUILT[c_s] = build_bass(c_s)
    nc = _BUILT[c_s]

    in_maps = prepare_in_maps(inp, c_s)
    res = run_bass_kernel_spmd(nc, in_maps, list(range(NCORES)))
    out = np.concatenate(
        [np.ascontiguousarray(res.results[i]["out"].T) for i in range(NCORES)],
        axis=0)
    return out.astype(np.float32)


# revision 10
# speedup vs baseline: 1.4487x; 1.2406x over previous
"""Trainium2 Bass kernel for DescriptorNetwork (Roost-style GNN message passing).

Structure exploited (verified at runtime in kernel()):
  - N = C*K nodes, K=5 elements per crystal, edges = all-pairs within crystal
  - self_fea_idx = repeat(arange(N), 5), nbr_fea_idx = per-crystal tile,
    cry_elem_idx = repeat(arange(C), 5)
  => every gather is a strided/broadcast access pattern; every segment
     reduction is over 5 contiguous elements.

Sharding: 1250 crystals per core x 8 cores, fully data parallel, no
collectives.  Everything on-chip is feature-major (features on SBUF
partitions, nodes/edges along the free dimension).

v2 layout (vs the first working version):
  - elem_fea is transposed host-side -> no on-chip transposes for embedding
  - x keeps an fp32 master (xT) plus a bf16 mirror (xb); catT is built once
    per layer into a persistent bf16 cache and reused by both passes
  - Lrelu activations are split between the scalar engine (ACT) and the
    vector engine (mul+max 2-op form; biases are all zero per the spec)
  - gate broadcast to 64 partitions via a K=1 ones-matmul on the PE from a
    bf16 copy of the normalized gate (no DRAM bounce)
  - pooling matmuls run in bf16 from xb
  - output stays feature-major [64, c_s]; the host transposes
  - residual updates are chunked so the next layer starts while the tail of
    the current layer finishes (keeps the PE HAM clock warm)
"""

import numpy as np
import ml_dtypes
from contextlib import ExitStack

import concourse.bass as bass
import concourse.tile as tile
from concourse import mybir
from concourse.alu_op_type import AluOpType
from concourse.bass_utils import run_bass_kernel_spmd

FP32 = mybir.dt.float32
BF16 = mybir.dt.bfloat16
AF = mybir.ActivationFunctionType

# Model constants (hardcoded per problem spec)
C_TOT = 10000
K = 5
N_TOT = C_TOT * K
EMB = 200
F = 64
L = 3
H = 3
HID = 256
NCORES = 8

C_S = C_TOT // NCORES          # 1250 crystals per core
GCOLS = 250                    # gate buffer: 250 edges (10 crystals) per row
WCOLS = 50                     # node buffer: 50 nodes (10 crystals) per row
TE = 500                       # edge tile (100 nodes, 20 crystals)
TN = 500                       # node tile for pooling
TNE = 512                      # node tile for embedding
RES_GROUP = 16                 # pass-2 tiles per residual-update chunk

# Lrelu engine per (head, chunk) slot: 's'=scalar ACT, 'v'=vector 2-op
ENG_P1 = ["s", "v", "s", "s", "v", "s"]
ENG_P2 = ["s", "v", "s", "s", "v", "s"]


def _tiles(total, size):
    out, o = [], 0
    while o < total:
        out.append((o, min(size, total - o)))
        o += size
    return out


def _split_multiwaits(nc):
    """Walrus in this container encodes at most one on_wait per instruction;
    Tile emits several.  Split extras into preceding wait-only instructions."""
    n_split = 0
    for bb in nc.main_func.blocks:
        new = []
        for inst in bb.instructions:
            si = getattr(inst, "sync_info", None)
            waits = list(si.on_wait) if (si is not None and si.on_wait) else []
            if len(waits) > 1:
                for w in waits[:-1]:
                    ev = mybir.InstEventSemaphore(
                        name=f"{inst.name}-w{n_split}",
                        ins=[], outs=[],
                        sync_info=mybir.SyncInfo(on_wait=[w], on_update=[]),
                    )
                    ev.engine = inst.engine
                    new.append(ev)
                    n_split += 1
                si.on_wait = [waits[-1]]
            new.append(inst)
        bb.instructions[:] = new
    return n_split


def build_bass(c_s=C_S, split_waits=True):
    """Build the per-core Bass program (same program on all 8 cores).
    Assumes all hidden-layer biases (b1) are zero (guaranteed by the spec
    fills; checked host-side with fallback)."""
    n_s, e_s = c_s * K, c_s * K * K
    assert e_s % GCOLS == 0 and n_s % WCOLS == 0
    grows, wrows = e_s // GCOLS, n_s // WCOLS

    nc = bass.Bass()

    # ---- DRAM parameters (host pre-packs layouts; see _pack_weights) ----
    d_feaT = nc.declare_dram_parameter("feaT", [128, 2 * n_s], FP32, isOutput=False)
    d_ew = nc.declare_dram_parameter("elem_weights", [n_s], FP32, isOutput=False)
    d_embW = nc.declare_dram_parameter("embW", [128, 2 * 63], FP32, isOutput=False)
    d_embB = nc.declare_dram_parameter("embB", [63, 1], FP32, isOutput=False)
    d_gW1 = nc.declare_dram_parameter("gW1", [128, L * 2 * H * 2 * 128], BF16, isOutput=False)
    d_gW2m = nc.declare_dram_parameter("gW2m", [128, L * H * 2 * 64], BF16, isOutput=False)
    d_gw2g = nc.declare_dram_parameter("gw2g", [128, L * H * 2], BF16, isOutput=False)
    d_gxb = nc.declare_dram_parameter("gxb", [64, L], FP32, isOutput=False)
    d_pw = nc.declare_dram_parameter("pw", [grows, L * H], FP32, isOutput=False)
    d_b2g = nc.declare_dram_parameter("b2g", [grows, L * H], FP32, isOutput=False)
    d_cW1 = nc.declare_dram_parameter("cW1", [64, 2 * H * 2 * 128], BF16, isOutput=False)
    d_cW2m = nc.declare_dram_parameter("cW2m", [128, H * 2 * 64], BF16, isOutput=False)
    d_cw2g = nc.declare_dram_parameter("cw2g", [128, H * 2], BF16, isOutput=False)
    d_cxb = nc.declare_dram_parameter("cxb", [64, 1], FP32, isOutput=False)
    d_cpw = nc.declare_dram_parameter("cpw", [wrows, H], FP32, isOutput=False)
    d_cb2g = nc.declare_dram_parameter("cb2g", [wrows, H], FP32, isOutput=False)
    d_out = nc.declare_dram_parameter("out", [F, c_s], FP32, isOutput=True)

    with ExitStack() as ctx:
        tc = ctx.enter_context(tile.TileContext(nc))
        per = ctx.enter_context(tc.tile_pool(name="persist", bufs=1))
        dram = ctx.enter_context(tc.tile_pool(name="dram", bufs=1, space="DRAM"))
        gdram = dram.tile([H, e_s], BF16, tag="gdram", name="gdram")
        cdram = dram.tile([H, n_s], BF16, tag="cdram", name="cdram")

        # ---- persistent SBUF ----
        xT = per.tile([F, n_s], FP32, tag="xT", name="xT")
        xb = per.tile([F, n_s], BF16, tag="xb", name="xb")
        catc = per.tile([128, e_s], BF16, tag="catc", name="catc")
        embW_s = per.tile([128, 2, 63], FP32, tag="embW", name="embW")
        embB_s = per.tile([63, 1], FP32, tag="embB", name="embB")
        gW1_s = per.tile([128, L, 2, H, 2, 128], BF16, tag="gW1", name="gW1")
        gW2m_s = per.tile([128, L, H, 2, 64], BF16, tag="gW2m", name="gW2m")
        gw2g_s = per.tile([128, L, H, 2], BF16, tag="gw2g", name="gw2g")
        gxb_s = per.tile([64, L], FP32, tag="gxb", name="gxb")
        pw_s = per.tile([grows, L * H], FP32, tag="pw", name="pw")
        b2g_s = per.tile([grows, L * H], FP32, tag="b2g", name="b2g")
        cW1_s = per.tile([64, 2, H, 2, 128], BF16, tag="cW1", name="cW1")
        cW2m_s = per.tile([128, H, 2, 64], BF16, tag="cW2m", name="cW2m")
        cw2g_s = per.tile([128, H, 2], BF16, tag="cw2g", name="cw2g")
        cxb_s = per.tile([64, 1], FP32, tag="cxb", name="cxb")
        cpw_s = per.tile([wrows, H], FP32, tag="cpw", name="cpw")
        cb2g_s = per.tile([wrows, H], FP32, tag="cb2g", name="cb2g")
        lnw_s = per.tile([wrows, WCOLS], FP32, tag="lnw", name="lnw")
        lnwe_s = per.tile([grows, GCOLS], FP32, tag="lnwe", name="lnwe")
        wbuf_s = per.tile([wrows, WCOLS], FP32, tag="wbuf", name="wbuf")
        # gate logit/softmax buffers, graph layers: [125, 3, 250]
        glog = per.tile([grows, H, GCOLS], FP32, tag="glog", name="glog")
        gexp = per.tile([grows, H, GCOLS], FP32, tag="gexp", name="gexp")
        gn3 = per.tile([grows, H, GCOLS], FP32, tag="gn3", name="gn3")
        gb3 = per.tile([grows, H, GCOLS], BF16, tag="gb3", name="gb3")
        lnw3 = per.tile([grows, H, GCOLS], FP32, tag="lnw3", name="lnw3")
        ssum = per.tile([grows, H, WCOLS], FP32, tag="ssum", name="ssum")
        rb3 = per.tile([grows, H, WCOLS], FP32, tag="rb3", name="rb3")
        # pooling buffers: [125, 3, 50]
        clog = per.tile([wrows, H, WCOLS], FP32, tag="clog", name="clog")
        cexp = per.tile([wrows, H, WCOLS], FP32, tag="cexp", name="cexp")
        cn3 = per.tile([wrows, H, WCOLS], FP32, tag="cn3", name="cn3")
        cb3 = per.tile([wrows, H, WCOLS], BF16, tag="cb3", name="cb3")
        lnwc3 = per.tile([wrows, H, WCOLS], FP32, tag="lnwc3", name="lnwc3")
        csum = per.tile([wrows, H, WCOLS // K], FP32, tag="csum", name="csum")
        crb = per.tile([wrows, H, WCOLS // K], FP32, tag="crb", name="crb")
        outsum = per.tile([F, c_s], FP32, tag="outsum", name="outsum")

        # ---- load weights / constants ----
        nc.sync.dma_start(embW_s[:], d_embW[:].rearrange("p (c f) -> p c f", c=2))
        nc.sync.dma_start(embB_s[:], d_embB[:])
        nc.sync.dma_start(gW1_s[:], d_gW1[:].rearrange(
            "p (l m h c v) -> p l m h c v", l=L, m=2, h=H, c=2))
        nc.sync.dma_start(gW2m_s[:], d_gW2m[:].rearrange(
            "p (l h c f) -> p l h c f", l=L, h=H, c=2))
        nc.sync.dma_start(gw2g_s[:], d_gw2g[:].rearrange(
            "p (l h c) -> p l h c", l=L, h=H))
        nc.sync.dma_start(gxb_s[:], d_gxb[:])
        nc.sync.dma_start(pw_s[:], d_pw[:])
        nc.sync.dma_start(b2g_s[:], d_b2g[:])
        nc.sync.dma_start(cW1_s[:], d_cW1[:].rearrange(
            "p (m h c v) -> p m h c v", m=2, h=H, c=2))
        nc.sync.dma_start(cW2m_s[:], d_cW2m[:].rearrange(
            "p (h c f) -> p h c f", h=H, c=2))
        nc.sync.dma_start(cw2g_s[:], d_cw2g[:].rearrange("p (h c) -> p h c", h=H))
        nc.sync.dma_start(cxb_s[:], d_cxb[:])
        nc.sync.dma_start(cpw_s[:], d_cpw[:])
        nc.sync.dma_start(cb2g_s[:], d_cb2g[:])

        nc.sync.dma_start(xT[63:64, :], d_ew[:].unsqueeze(0))
        nc.sync.dma_start(wbuf_s[:], d_ew[:].rearrange("(r c) -> r c", r=wrows))
        nc.scalar.activation(lnw_s[:], wbuf_s[:], AF.Ln)
        # edge-expanded ln(w): lnw_e[p, c, i, j] = lnw[p, c, j]
        nc.vector.tensor_copy(
            lnwe_s[:].rearrange("p (c i j) -> p c i j", i=K, j=K),
            lnw_s[:].rearrange("p (c j) -> p c j", j=K)
            .unsqueeze(2).broadcast_to([wrows, WCOLS // K, K, K]))

        def lrelu(eng, out_ap, in_ap, tmp_pool, tshape):
            if eng == "s":
                nc.scalar.activation(out_ap, in_ap, AF.Lrelu, alpha=0.01)
            else:
                tmp = tmp_pool.tile(tshape, BF16, tag="lrt", name="lrt")
                tsz = [s for s in in_ap.shape]
                tv = tmp[tuple(slice(0, s) for s in tsz)]
                nc.vector.tensor_scalar_mul(tv, in_ap, 0.01)
                nc.vector.tensor_tensor(out_ap, in_ap, tv, op=AluOpType.max)

        # ---- embedding: xT[0:63] = (elem_fea @ embW + embB)^T, xb = bf16 ----
        with tc.tile_pool(name="emb_sb", bufs=3) as embp, \
             tc.tile_pool(name="emb_ps", bufs=2, space="PSUM") as emb_ps:
            for n0, tn in _tiles(n_s, TNE):
                stage = embp.tile([128, 2, TNE], FP32, tag="stage", name="stage")
                for c in range(2):
                    nc.sync.dma_start(
                        stage[:, c, :tn],
                        d_feaT[:, c * n_s + n0:c * n_s + n0 + tn])
                emb_o = emb_ps.tile([63, TNE], FP32, tag="emb_o", name="emb_o")
                nc.tensor.matmul(emb_o[:, :tn], embW_s[:, 0, :], stage[:, 0, :tn],
                                 start=True, stop=False)
                nc.tensor.matmul(emb_o[:, :tn], embW_s[:, 1, :], stage[:, 1, :tn],
                                 start=False, stop=True)
                nc.scalar.activation(xT[0:63, n0:n0 + tn], emb_o[:, :tn],
                                     AF.Identity, bias=embB_s[:])
                nc.vector.tensor_copy(xb[:, n0:n0 + tn], xT[:, n0:n0 + tn])

        # ---- graph message-passing layers ----
        for l in range(L):
            etiles = _tiles(e_s, TE)

            # ----- PASS 1: build catc; gate hidden -> logits -> glog -----
            with tc.tile_pool(name="p1_hg", bufs=4) as hgp, \
                 tc.tile_pool(name="p1_t", bufs=3) as tpp, \
                 tc.tile_pool(name="p1_gs", bufs=3) as gsp, \
                 tc.tile_pool(name="p1_z", bufs=3, space="PSUM") as zp, \
                 tc.tile_pool(name="p1_g", bufs=2, space="PSUM") as gp:
                for e0, te in etiles:
                    nn0, tnn = e0 // K, te // K
                    tcc = te // (K * K)
                    nc.gpsimd.tensor_copy(
                        catc[0:64, e0:e0 + te].rearrange("p (n r) -> p n r", r=K),
                        xb[:, nn0:nn0 + tnn].unsqueeze(2).broadcast_to([F, tnn, K]))
                    nc.gpsimd.tensor_copy(
                        catc[64:128, e0:e0 + te].rearrange(
                            "p (c r j) -> p c r j", r=K, j=K),
                        xb[:, nn0:nn0 + tnn].rearrange("p (c j) -> p c j", j=K)
                        .unsqueeze(2).broadcast_to([F, tcc, K, K]))
                    gt3 = gp.tile([96, 512], FP32, tag="g3", name="g3")
                    zts, hgs = [], []
                    for h in range(H):
                        zt = zp.tile([128, 2, 512], FP32, tag="z", name="z")
                        for c in range(2):
                            nc.tensor.matmul(zt[:, c, :te], gW1_s[:, l, 0, h, c, :],
                                             catc[:, e0:e0 + te],
                                             start=True, stop=True)
                        zts.append(zt)
                    for h in range(H):
                        hg = hgp.tile([128, 2, TE], BF16, tag="hg", name="hg")
                        eng = "v" if ((e0 // TE) * H + h) % 4 == 3 else "s"
                        lrelu(eng, hg[:, :, :te], zts[h][:, :, :te], tpp,
                              [128, 2, TE])
                        hgs.append(hg)
                    for h in range(H):
                        for c in range(2):
                            nc.tensor.matmul(gt3[32 * h:32 * h + 1, :te],
                                             gw2g_s[:, l, h, c:c + 1],
                                             hgs[h][:, c, :te],
                                             start=(c == 0), stop=(c == 1))
                    r0 = e0 // GCOLS
                    gs3 = gsp.tile([96, TE], FP32, tag="gs", name="gs")
                    nc.vector.tensor_copy(gs3[:, :te], gt3[0:96, :te])
                    for h in range(H):
                        nc.sync.dma_start(glog[r0:r0 + te // GCOLS, h, :],
                                          gs3[32 * h:32 * h + 1, :te])

            # ----- segment softmax for all 3 heads of layer l -----
            for h in range(H):
                lh = l * H + h
                nc.vector.tensor_scalar(lnw3[:, h, :], lnwe_s[:],
                                        pw_s[:, lh:lh + 1], b2g_s[:, lh:lh + 1],
                                        op0=AluOpType.mult, op1=AluOpType.add)
            nc.gpsimd.tensor_tensor(gexp[:], glog[:], lnw3[:], op=AluOpType.add)
            nc.scalar.activation(gexp[:], gexp[:], AF.Exp)
            nc.vector.tensor_reduce(ssum[:], gexp[:].rearrange(
                "p h (s j) -> p h s j", j=K), axis=mybir.AxisListType.X,
                op=AluOpType.add)
            nc.vector.tensor_scalar_add(ssum[:], ssum[:], 1e-10)
            nc.vector.reciprocal(rb3[:], ssum[:])
            nc.vector.tensor_tensor(
                gn3[:].rearrange("p h (s j) -> p h s j", j=K),
                gexp[:].rearrange("p h (s j) -> p h s j", j=K),
                rb3[:].unsqueeze(3).broadcast_to([grows, H, WCOLS, K]),
                op=AluOpType.mult)
            nc.gpsimd.tensor_copy(gb3[:], gn3[:])
            for h in range(H):
                nc.sync.dma_start(gdram[h], gb3[:, h, :])

            # ----- PASS 2: msg hidden -> W2 -> gate-weighted segsum -> resid ----
            with tc.tile_pool(name="p2_hm", bufs=4) as hmp, \
                 tc.tile_pool(name="p2_t", bufs=3) as tpp, \
                 tc.tile_pool(name="p2_bc", bufs=4) as bcs, \
                 tc.tile_pool(name="p2_mw", bufs=2) as mwp, \
                 tc.tile_pool(name="p2_hs", bufs=2) as hsp, \
                 tc.tile_pool(name="p2_z", bufs=3, space="PSUM") as zp, \
                 tc.tile_pool(name="p2_w", bufs=2, space="PSUM") as wp:
                hsum_g = None
                g_nn0 = 0
                for ti, (e0, te) in enumerate(etiles):
                    nn0, tnn = e0 // K, te // K
                    if hsum_g is None:
                        hsum_g = hsp.tile([F, RES_GROUP * (TE // K)], FP32,
                                          tag="hsg", name="hsg")
                        g_nn0 = nn0
                    grow = []
                    for h in range(H):
                        bc = bcs.tile([64, TE], BF16, tag="bc", name="bc")
                        nc.sync.dma_start(
                            bc[:, :te],
                            gdram[h, e0:e0 + te].unsqueeze(0).unsqueeze(0)
                            .broadcast_to([1, 64, te]).squeeze(0))
                        grow.append(bc)
                    msgw = mwp.tile([F, TE // K, H, K], BF16, tag="mw", name="mw")
                    zts, hms = [], []
                    for h in range(H):
                        zt = zp.tile([128, 2, 512], FP32, tag="z", name="z")
                        for c in range(2):
                            nc.tensor.matmul(zt[:, c, :te], gW1_s[:, l, 1, h, c, :],
                                             catc[:, e0:e0 + te],
                                             start=True, stop=True)
                        zts.append(zt)
                    for h in range(H):
                        hm = hmp.tile([128, 2, TE], BF16, tag="hm", name="hm")
                        eng = "v" if (ti * H + h) % 4 == 1 else "s"
                        lrelu(eng, hm[:, :, :te], zts[h][:, :, :te], tpp,
                              [128, 2, TE])
                        hms.append(hm)
                    for h in range(H):
                        w2 = wp.tile([64, 512], FP32, tag="w2", name="w2")
                        nc.tensor.matmul(w2[:, :te], gW2m_s[:, l, h, 0, :],
                                         hms[h][:, 0, :te], start=True, stop=False)
                        nc.tensor.matmul(w2[:, :te], gW2m_s[:, l, h, 1, :],
                                         hms[h][:, 1, :te], start=False, stop=True)
                        nc.vector.tensor_tensor(
                            msgw[:, :tnn, h, :],
                            w2[:, :te].rearrange("p (n r) -> p n r", r=K),
                            grow[h][:, :te].rearrange("p (n r) -> p n r", r=K),
                            op=AluOpType.mult)
                    nc.vector.tensor_reduce(
                        hsum_g[:, nn0 - g_nn0:nn0 - g_nn0 + tnn],
                        msgw[:, :tnn, :, :],
                        axis=mybir.AxisListType.XY, op=AluOpType.add)
                    # residual-update chunk boundary
                    if ti % RES_GROUP == RES_GROUP - 1 or ti == len(etiles) - 1:
                        gn = nn0 + tnn - g_nn0
                        nc.gpsimd.tensor_tensor(
                            hsum_g[:, :gn], hsum_g[:, :gn],
                            xT[:, g_nn0:g_nn0 + gn], op=AluOpType.add)
                        nc.scalar.activation(xT[:, g_nn0:g_nn0 + gn],
                                             hsum_g[:, :gn], AF.Identity,
                                             bias=gxb_s[:, l:l + 1])
                        nc.gpsimd.tensor_copy(xb[:, g_nn0:g_nn0 + gn],
                                              xT[:, g_nn0:g_nn0 + gn])
                        hsum_g = None

        # ---- crystal pooling ----
        ntiles = _tiles(n_s, TN)
        # PASS 1: gate logits
        with tc.tile_pool(name="c1_hg", bufs=4) as hgp, \
             tc.tile_pool(name="c1_t", bufs=3) as tpp, \
             tc.tile_pool(name="c1_gs", bufs=3) as gsp, \
             tc.tile_pool(name="c1_z", bufs=3, space="PSUM") as zp, \
             tc.tile_pool(name="c1_g", bufs=2, space="PSUM") as gp:
            for n0, tn in ntiles:
                gt3 = gp.tile([96, 512], FP32, tag="g3", name="g3")
                zts, hgs = [], []
                for h in range(H):
                    zt = zp.tile([128, 2, 512], FP32, tag="z", name="z")
                    for c in range(2):
                        nc.tensor.matmul(zt[:, c, :tn], cW1_s[:, 0, h, c, :],
                                         xb[:, n0:n0 + tn], start=True, stop=True)
                    zts.append(zt)
                for h in range(H):
                    hg = hgp.tile([128, 2, TN], BF16, tag="hg", name="hg")
                    eng = "v" if ((n0 // TN) * H + h) % 4 == 3 else "s"
                    lrelu(eng, hg[:, :, :tn], zts[h][:, :, :tn], tpp, [128, 2, TN])
                    hgs.append(hg)
                for h in range(H):
                    for c in range(2):
                        nc.tensor.matmul(gt3[32 * h:32 * h + 1, :tn],
                                         cw2g_s[:, h, c:c + 1], hgs[h][:, c, :tn],
                                         start=(c == 0), stop=(c == 1))
                r0 = n0 // WCOLS
                gs3 = gsp.tile([96, TN], FP32, tag="gs", name="gs")
                nc.vector.tensor_copy(gs3[:, :tn], gt3[0:96, :tn])
                for h in range(H):
                    nc.sync.dma_start(clog[r0:r0 + tn // WCOLS, h, :],
                                      gs3[32 * h:32 * h + 1, :tn])

        # pooling softmax (segments = 5 nodes of each crystal)
        for h in range(H):
            nc.vector.tensor_scalar(lnwc3[:, h, :], lnw_s[:],
                                    cpw_s[:, h:h + 1], cb2g_s[:, h:h + 1],
                                    op0=AluOpType.mult, op1=AluOpType.add)
        nc.gpsimd.tensor_tensor(cexp[:], clog[:], lnwc3[:], op=AluOpType.add)
        nc.scalar.activation(cexp[:], cexp[:], AF.Exp)
        nc.vector.tensor_reduce(csum[:], cexp[:].rearrange(
            "p h (s j) -> p h s j", j=K), axis=mybir.AxisListType.X,
            op=AluOpType.add)
        nc.vector.tensor_scalar_add(csum[:], csum[:], 1e-10)
        nc.vector.reciprocal(crb[:], csum[:])
        nc.vector.tensor_tensor(
            cn3[:].rearrange("p h (s j) -> p h s j", j=K),
            cexp[:].rearrange("p h (s j) -> p h s j", j=K),
            crb[:].unsqueeze(3).broadcast_to([wrows, H, WCOLS // K, K]),
            op=AluOpType.mult)
        nc.gpsimd.tensor_copy(cb3[:], cn3[:])
        for h in range(H):
            nc.sync.dma_start(cdram[h], cb3[:, h, :])

        # PASS 2: messages
        with tc.tile_pool(name="c2_hm", bufs=4) as hmp, \
             tc.tile_pool(name="c2_t", bufs=3) as tpp, \
             tc.tile_pool(name="c2_bc", bufs=4) as bcs, \
             tc.tile_pool(name="c2_mw", bufs=2) as mwp, \
             tc.tile_pool(name="c2_z", bufs=3, space="PSUM") as zp, \
             tc.tile_pool(name="c2_w", bufs=2, space="PSUM") as wp:
            for n0, tn in ntiles:
                cc0, tcc = n0 // K, tn // K
                grow = []
                for h in range(H):
                    bc = bcs.tile([64, TN], BF16, tag="bc", name="bc")
                    nc.sync.dma_start(
                        bc[:, :tn],
                        cdram[h, n0:n0 + tn].unsqueeze(0).unsqueeze(0)
                        .broadcast_to([1, 64, tn]).squeeze(0))
                    grow.append(bc)
                msgw = mwp.tile([F, TN // K, H, K], BF16, tag="mw", name="mw")
                zts, hms = [], []
                for h in range(H):
                    zt = zp.tile([128, 2, 512], FP32, tag="z", name="z")
                    for c in range(2):
                        nc.tensor.matmul(zt[:, c, :tn], cW1_s[:, 1, h, c, :],
                                         xb[:, n0:n0 + tn], start=True, stop=True)
                    zts.append(zt)
                for h in range(H):
                    hm = hmp.tile([128, 2, TN], BF16, tag="hm", name="hm")
                    eng = "v" if ((n0 // TN) * H + h) % 4 == 1 else "s"
                    lrelu(eng, hm[:, :, :tn], zts[h][:, :, :tn], tpp, [128, 2, TN])
                    hms.append(hm)
                for h in range(H):
                    w2 = wp.tile([64, 512], FP32, tag="w2", name="w2")
                    nc.tensor.matmul(w2[:, :tn], cW2m_s[:, h, 0, :],
                                     hms[h][:, 0, :tn], start=True, stop=False)
                    nc.tensor.matmul(w2[:, :tn], cW2m_s[:, h, 1, :],
                                     hms[h][:, 1, :tn], start=False, stop=True)
                    nc.vector.tensor_tensor(
                        msgw[:, :tcc, h, :],
                        w2[:, :tn].rearrange("p (n r) -> p n r", r=K),
                        grow[h][:, :tn].rearrange("p (n r) -> p n r", r=K),
                        op=AluOpType.mult)
                nc.vector.tensor_reduce(
                    outsum[:, cc0:cc0 + tcc], msgw[:, :tcc, :, :],
                    axis=mybir.AxisListType.XY, op=AluOpType.add)

        # out = outsum + cxb, store feature-major; the host transposes
        nc.scalar.activation(outsum[:], outsum[:], AF.Identity, bias=cxb_s[:])
        nc.sync.dma_start(d_out[:], outsum[:])

    if split_waits:
        _split_multiwaits(nc)
    return nc


def _pack_weights(inp, grows, wrows):
    """Host-side packing of (replicated) weights into SBUF-ready layouts."""
    f32 = np.float32
    bf16 = ml_dtypes.bfloat16
    gW1 = np.zeros((128, L, 2, H, 2, 128), f32)
    for l in range(L):
        for h in range(H):
            for c in range(2):
                sl = slice(c * 128, (c + 1) * 128)
                gW1[:, l, 0, h, c, :] = inp["g_gate_W1"][l, h][:, sl]
                gW1[:, l, 1, h, c, :] = inp["g_msg_W1"][l, h][:, sl]
    gW2m = np.zeros((128, L, H, 2, 64), f32)
    gw2g = np.zeros((128, L, H, 2), f32)
    for l in range(L):
        for h in range(H):
            for c in range(2):
                sl = slice(c * 128, (c + 1) * 128)
                gW2m[:, l, h, c, :] = inp["g_msg_W2"][l, h][sl, :] / 3.0
                gw2g[:, l, h, c] = inp["g_gate_W2"][l, h][sl, 0]
    gxb = (np.sum(inp["g_msg_b2"], axis=1).T / 3.0).astype(f32)      # [64, L]
    pw = np.tile(np.asarray(inp["g_pow"], f32).reshape(1, L * H), (grows, 1))
    b2g = np.tile(np.asarray(inp["g_gate_b2"], f32).reshape(1, L * H), (grows, 1))

    cW1 = np.zeros((64, 2, H, 2, 128), f32)
    cW2m = np.zeros((128, H, 2, 64), f32)
    cw2g = np.zeros((128, H, 2), f32)
    for h in range(H):
        for c in range(2):
            sl = slice(c * 128, (c + 1) * 128)
            cW1[:, 0, h, c, :] = inp["c_gate_W1"][h][:, sl]
            cW1[:, 1, h, c, :] = inp["c_msg_W1"][h][:, sl]
            cW2m[:, h, c, :] = inp["c_msg_W2"][h][sl, :] / 3.0
            cw2g[:, h, c] = inp["c_gate_W2"][h][sl, 0]
    cxb = (np.sum(inp["c_msg_b2"], axis=0) / 3.0).astype(f32).reshape(64, 1)
    cpw = np.tile(np.asarray(inp["c_pow"], f32).reshape(1, H), (wrows, 1))
    cb2g = np.tile(np.asarray(inp["c_gate_b2"], f32).reshape(1, H), (wrows, 1))

    return dict(
        embW=np.pad(np.asarray(inp["emb_W"], f32), ((0, 56), (0, 0)))
        .reshape(2, 128, 63).transpose(1, 0, 2).reshape(128, 2 * 63).copy(),
        embB=np.asarray(inp["emb_b"], f32).reshape(63, 1),
        gW1=gW1.reshape(128, -1).astype(bf16),
        gW2m=gW2m.reshape(128, -1).astype(bf16),
        gw2g=gw2g.reshape(128, -1).astype(bf16),
        gxb=gxb, pw=pw, b2g=b2g,
        cW1=cW1.reshape(64, -1).astype(bf16),
        cW2m=cW2m.reshape(128, -1).astype(bf16),
        cw2g=cw2g.reshape(128, -1).astype(bf16),
        cxb=cxb, cpw=cpw, cb2g=cb2g,
    )


def prepare_in_maps(inp, c_s):
    """Build the 8 per-core input maps (weights replicated, data sharded)."""
    n_s = c_s * K
    grows = (c_s * K * K) // GCOLS
    wrows = n_s // WCOLS
    wmap = _pack_weights(inp, grows, wrows)

    fea = np.asarray(inp["elem_fea"], np.float32)
    n_tot = fea.shape[0]
    feaT = np.zeros((128, 2, n_tot), np.float32)
    ft = np.ascontiguousarray(fea.T)               # [200, N]
    feaT[:, 0, :] = ft[0:128]
    feaT[0:EMB - 128, 1, :] = ft[128:EMB]
    ew = np.asarray(inp["elem_weights"], np.float32).reshape(-1)

    in_maps = []
    for i in range(NCORES):
        m = dict(wmap)
        m["feaT"] = np.ascontiguousarray(
            feaT[:, :, i * n_s:(i + 1) * n_s]).reshape(128, 2 * n_s)
        m["elem_weights"] = ew[i * n_s:(i + 1) * n_s].copy()
        in_maps.append(m)
    return in_maps


def _check_structure(inp):
    n = inp["elem_fea"].shape[0]
    c = n // K
    e = inp["self_fea_idx"].shape[0]
    if e != c * K * K:
        return False
    if int(inp["n_crystals"]) != c:
        return False
    # all hidden biases must be zero (they are, per the spec fills)
    for k in ("g_gate_b1", "g_msg_b1", "c_gate_b1", "c_msg_b1"):
        if not np.all(np.asarray(inp[k]) == 0):
            return False
    self_ref = np.repeat(np.arange(n, dtype=np.int64), K)
    ar = np.arange(e, dtype=np.int64)
    nbr_ref = (ar // (K * K)) * K + (ar % K)
    cry_ref = np.repeat(np.arange(c, dtype=np.int64), K)
    return (np.array_equal(np.asarray(inp["self_fea_idx"]), self_ref)
            and np.array_equal(np.asarray(inp["nbr_fea_idx"]), nbr_ref)
            and np.array_equal(np.asarray(inp["cry_elem_idx"]), cry_ref))


def _reference_numpy(inp):
    """Fallback (never used when structure+zero-bias checks pass)."""
    def simple(hh, W1, b1, W2, b2):
        t = hh @ W1 + b1
        t = np.where(t > 0, t, 0.01 * t)
        return t @ W2 + b2

    def attn(fea, weights, index, nseg, gW1, gb1, gW2, gb2, mW1, mb1, mW2, mb2, p):
        gate = simple(fea, gW1, gb1, gW2, gb2)
        gmax = np.full((nseg, 1), -np.inf, np.float32)
        np.maximum.at(gmax, index[:, 0] if index.ndim > 1 else index, gate)
        gate = gate - gmax[index]
        gate = weights ** p * np.exp(gate)
        gsum = np.zeros((nseg, 1), np.float32)
        np.add.at(gsum, index, gate)
        gate = gate / (gsum[index] + 1e-10)
        msg = simple(fea, mW1, mb1, mW2, mb2)
        out = np.zeros((nseg, msg.shape[1]), np.float32)
        np.add.at(out, index, gate * msg)
        return out

    inp = {k: np.asarray(v) for k, v in inp.items()}
    n = inp["elem_fea"].shape[0]
    x = np.concatenate([inp["elem_fea"] @ inp["emb_W"] + inp["emb_b"],
                        inp["elem_weights"]], axis=1)
    w_nbr = inp["elem_weights"][inp["nbr_fea_idx"]]
    si, ni = inp["self_fea_idx"], inp["nbr_fea_idx"]
    for l in range(L):
        cat = np.concatenate([x[si], x[ni]], axis=1)
        heads = [attn(cat, w_nbr, si, n,
                      inp["g_gate_W1"][l, h], inp["g_gate_b1"][l, h],
                      inp["g_gate_W2"][l, h], inp["g_gate_b2"][l, h],
                      inp["g_msg_W1"][l, h], inp["g_msg_b1"][l, h],
                      inp["g_msg_W2"][l, h], inp["g_msg_b2"][l, h],
                      inp["g_pow"][l, h]) for h in range(H)]
        x = np.mean(heads, axis=0) + x
    ci = inp["cry_elem_idx"]
    cn = int(inp["n_crystals"])
    heads = [attn(x, inp["elem_weights"], ci, cn,
                  inp["c_gate_W1"][h], inp["c_gate_b1"][h],
                  inp["c_gate_W2"][h], inp["c_gate_b2"][h],
                  inp["c_msg_W1"][h], inp["c_msg_b1"][h],
                  inp["c_msg_W2"][h], inp["c_msg_b2"][h],
                  inp["c_pow"][h]) for h in range(H)]
    return np.mean(heads, axis=0).astype(np.float32)


_BUILT = {}


def kernel(**inputs):
    inp = {k: np.asarray(v) if not np.isscalar(v) else v for k, v in inputs.items()}
    if not _check_structure(inp):
        return _reference_numpy(inp)

    n_tot = inp["elem_fea"].shape[0]
    c_tot = n_tot // K
    assert c_tot % NCORES == 0
    c_s = c_tot // NCORES

    if c_s not in _BUILT:
        _BUILT[c_s] = build_bass(c_s)
    nc = _BUILT[c_s]

    in_maps = prepare_in_maps(inp, c_s)
    res = run_bass_kernel_spmd(nc, in_maps, list(range(NCORES)))
    out = np.concatenate(
        [np.ascontiguousarray(res.results[i]["out"].T) for i in range(NCORES)],
        axis=0)
    return out.astype(np.float32)
